# revision 32
# baseline (speedup 1.0000x reference)
"""DiffNet++ (GATv2 diffusion + gamma gating + dot-product prediction) on 8
Trainium2 NeuronCores via Bass/Tile.  v2 — DVE-light edge pipeline.

Strategy (dst-range edge sharding, one SPMD program):
  - Users/items row-sharded: users 98 tiles (12544 rows)/core, items 49 tiles
    (6272 rows)/core. Each GAT edge belongs to the core owning its dst.
  - Projections in f16, packed into 128-col tables so dma_gather rows are
    exactly 256B: fsU = [fs_rate | fs_tr] (AllGathered, UPAD rows),
    fsI = [fs_rb | fd_rate] (AllGathered, IPAD rows), fdU = [fd_rb | fd_tr]
    (local US rows).
  - Per edge slot, gather BOTH fs[src] (banked, from the global table) and
    fd[dst] (single-range, from the local table). x = fs+fd, leaky, e = a.x,
    z = exp(e) — batched f16 vector ops, exp on the scalar engine.
  - Segment softmax without max subtraction (logits ~1e-2): out[v] =
    (sum_e z_e fs[src]) / (sum_e z_e) via one z-scaled one-hot matmul per
    sub-tile: onehot = (iota == dlc) * z built in ONE 4x tensor_scalar op;
    a 1.0 column memset into the gathered fs tile makes the denominator a
    free 65th matmul column.
  - Epilogue (gamma gating MLPs) batched: per-tile f16 transposes + split-W1
    PSUM-accumulated matmuls, vector work batched over 8 node tiles.
  - hu/hi concat tables built locally in f16 [*, 256] and AllGathered once.
  - Prediction: gather both sides per edge (512B rows); fused f16 dots.
"""
import sys

sys.path.insert(0, "/opt/trn_rl_repo")

from contextlib import ExitStack

import numpy as np

import concourse.bass as bass
import concourse.tile as tile
from concourse import bacc, mybir
from concourse.bass_utils import run_bass_kernel_spmd

N_CORES = 8
P = 128
BANK = 32768
GAT_SLOPE = 0.2
MLP_SLOPE = 0.01
F16 = mybir.dt.float16
F32 = mybir.dt.float32
I16 = mybir.dt.int16
NPF16 = np.dtype("float16")

Alu = mybir.AluOpType
Act = mybir.ActivationFunctionType


def _ceil(a, b):
    return -(-a // b)


# ---------------------------------------------------------------------------
# host-side preprocessing
# ---------------------------------------------------------------------------

class GatStruct:
    """Canonical (core-uniform) structure for one GAT graph's edges."""

    def __init__(self, name, src, dst, table_rows, shard_tiles):
        self.name = name
        self.nb = _ceil(table_rows, BANK)
        self.shard_tiles = shard_tiles
        S = shard_tiles * P
        self.S = S

        core = np.minimum(dst // S, N_CORES - 1)
        win = (dst - core * S) // P
        bank = src // BANK

        cnt = np.zeros((N_CORES, shard_tiles, self.nb), dtype=np.int64)
        np.add.at(cnt, (core, win, bank), 1)
        self.Kb = [max(1, int(_ceil(int(cnt[:, :, b].max()), P)))
                   for b in range(self.nb)]
        self.K = sum(self.Kb)
        self.WB = max(1, min(7, 144 // self.K))
        self.blocks = []
        t = shard_tiles
        while t > 0:
            wbi = min(self.WB, t)
            self.blocks.append(wbi)
            t -= wbi
        self.G_total = shard_tiles * self.K  # sub-tiles per core overall
        self.total_cols = self.G_total * P // 16

        order = np.lexsort((src, bank, win, core))
        src_s = src[order]
        dst_s = dst[order]
        core_s = core[order]
        win_s = win[order]
        bank_s = bank[order]

        self.idx16 = []    # fs gather: src - bank*BANK
        self.idxfd = []    # fd gather: dst - core*S (local row)
        self.dlc = []      # dst-local-in-window (-1 pad), [128, G_total] f32
        for c in range(N_CORES):
            sel = core_s == c
            csrc = src_s[sel]
            cdst = dst_s[sel]
            cwin = win_s[sel]
            cbank = bank_s[sel]
            key = cwin.astype(np.int64) * self.nb + cbank
            ids = np.zeros((self.G_total * P,), dtype=np.int16)
            idf = np.zeros((self.G_total * P,), dtype=np.int16)
            dl = np.full((self.G_total * P,), -1.0, dtype=np.float32)
            # slot layout: per block: [bank b: [window wo: Kb[b]*128 slots]]
            slot0 = 0
            w_base = 0
            for wbi in self.blocks:
                for b in range(self.nb):
                    for wo in range(wbi):
                        w = w_base + wo
                        e0 = np.searchsorted(key, w * self.nb + b, "left")
                        e1 = np.searchsorted(key, w * self.nb + b, "right")
                        n = e1 - e0
                        nsw = self.Kb[b] * P
                        assert n <= nsw, (name, c, w, b, n, nsw)
                        ids[slot0:slot0 + n] = (csrc[e0:e1] - b * BANK).astype(np.int16)
                        idf[slot0:slot0 + n] = (cdst[e0:e1] - c * S).astype(np.int16)
                        dl[slot0:slot0 + n] = (cdst[e0:e1] - (c * S + w * P)).astype(np.float32)
                        slot0 += nsw
                w_base += wbi
            assert slot0 == self.G_total * P
            cols = self.total_cols
            j = np.arange(self.G_total * P)

            def wrap(v):
                a = np.empty((16, cols), dtype=np.int16)
                a[j % 16, j // 16] = v
                return np.tile(a, (8, 1))

            self.idx16.append(wrap(ids))
            self.idxfd.append(wrap(idf))
            self.dlc.append(np.ascontiguousarray(
                dl.reshape(self.G_total, P).T))          # [128, G_total]


class PredStruct:
    """Canonical structure for prediction edges (pos+neg concatenated)."""

    def __init__(self, src, dst, u_rows, i_rows, block_edges):
        E = len(src)
        assert E % N_CORES == 0
        per_core = E // N_CORES
        self.per_core = per_core
        self.nbu = _ceil(u_rows, BANK)
        self.nbi = _ceil(i_rows, BANK)
        self.n_blocks = _ceil(per_core, block_edges)
        pairs = [(u_, i_) for u_ in range(self.nbu) for i_ in range(self.nbi)]
        self.pairs = pairs

        core = np.arange(E) // per_core
        blk = (np.arange(E) % per_core) // block_edges
        ub = src // BANK
        ib = dst // BANK
        cnt = np.zeros((N_CORES, self.n_blocks, self.nbu, self.nbi), dtype=np.int64)
        np.add.at(cnt, (core, blk, ub, ib), 1)
        self.Kp = {pq: max(1, int(_ceil(int(cnt[:, :, pq[0], pq[1]].max()), P)))
                   for pq in pairs}
        self.G_blk = sum(self.Kp[pq] for pq in pairs)
        self.G_total = self.G_blk * self.n_blocks

        self.idxu = []
        self.idxi = []
        self.slotmap = []
        for c in range(N_CORES):
            lo = c * per_core
            cs = src[lo:lo + per_core]
            cd = dst[lo:lo + per_core]
            iu = np.zeros((self.G_total * P,), dtype=np.int16)
            ii = np.zeros((self.G_total * P,), dtype=np.int16)
            smap = np.full((self.G_total * P,), -1, dtype=np.int64)
            for bi in range(self.n_blocks):
                b0 = bi * block_edges
                b1 = min(b0 + block_edges, per_core)
                bs, bd = cs[b0:b1], cd[b0:b1]
                bub, bib = bs // BANK, bd // BANK
                key = bub.astype(np.int64) * self.nbi + bib
                ordk = np.lexsort((bs, key))
                keys = key[ordk]
                off = bi * self.G_blk * P
                for pq in pairs:
                    kv = pq[0] * self.nbi + pq[1]
                    e0 = np.searchsorted(keys, kv, "left")
                    e1 = np.searchsorted(keys, kv, "right")
                    n = e1 - e0
                    npad = self.Kp[pq] * P
                    assert n <= npad
                    sel2 = ordk[e0:e1]
                    iu[off:off + n] = (bs[sel2] - pq[0] * BANK).astype(np.int16)
                    ii[off:off + n] = (bd[sel2] - pq[1] * BANK).astype(np.int16)
                    smap[off:off + n] = lo + b0 + sel2
                    off += npad
            cols = self.G_total * P // 16
            j = np.arange(self.G_total * P)
            au = np.empty((16, cols), dtype=np.int16)
            au[j % 16, j // 16] = iu
            ai = np.empty((16, cols), dtype=np.int16)
            ai[j % 16, j // 16] = ii
            self.idxu.append(np.tile(au, (8, 1)))
            self.idxi.append(np.tile(ai, (8, 1)))
            self.slotmap.append(smap)


# ---------------------------------------------------------------------------
# benchmarking helper (steady-state pipelined timing via PJRT)
# ---------------------------------------------------------------------------

def bench_pjrt(nc, in_maps, iters=3):
    """Time steady-state executions of the compiled program on the 8 cores."""
    import time as _time
    import jax
    from jax.sharding import Mesh, PartitionSpec
    from jax.experimental.shard_map import shard_map
    from concourse import bass2jax
    from concourse import mybir as _mb

    bass2jax.install_neuronx_cc_hook()
    partition_name = (nc.partition_id_tensor.name
                      if nc.partition_id_tensor else None)
    in_names, out_names, out_avals = [], [], []
    for alloc in nc.m.functions[0].allocations:
        if not isinstance(alloc, _mb.MemoryLocationSet):
            continue
        name = alloc.memorylocations[0].name
        if alloc.kind == "ExternalInput":
            if name != partition_name:
                in_names.append(name)
        elif alloc.kind == "ExternalOutput":
            out_names.append(name)
            out_avals.append(jax.core.ShapedArray(
                tuple(alloc.tensor_shape), _mb.dt.np(alloc.dtype)))
    n_params = len(in_names)
    zero_outs = [np.zeros(a.shape, a.dtype) for a in out_avals]
    all_names = in_names + out_names
    if partition_name is not None:
        all_names = all_names + [partition_name]

    def _body(*args):
        operands = list(args)
        if partition_name is not None:
            operands.append(bass2jax.partition_id_tensor())
        return tuple(bass2jax._bass_exec_p.bind(
            *operands, out_avals=tuple(out_avals),
            in_names=tuple(all_names), out_names=tuple(out_names),
            lowering_input_output_aliases=(), sim_require_finite=True,
            sim_require_nnan=True, nc=nc))

    devices = jax.devices()[:N_CORES]
    mesh = Mesh(np.asarray(devices), ("core",))
    nspec = n_params + len(out_names)
    f = jax.jit(shard_map(_body, mesh=mesh,
                          in_specs=(PartitionSpec("core"),) * nspec,
                          out_specs=(PartitionSpec("core"),) * len(out_names),
                          check_rep=False), keep_unused=True)
    from jax.sharding import NamedSharding
    sh = NamedSharding(mesh, PartitionSpec("core"))
    concat_in = [np.concatenate([np.asarray(m[nm]) for m in in_maps], axis=0)
                 for nm in in_names]
    concat_in += [np.concatenate([z] * N_CORES, axis=0) for z in zero_outs]
    dev_in = [jax.device_put(x, sh) for x in concat_in]
    times = []
    for i in range(iters):
        t0 = _time.time()
        outs = f(*dev_in)
        jax.block_until_ready(outs)
        times.append(_time.time() - t0)
    print(f"[bench] iter times: {[f'{t*1e3:.2f}ms' for t in times]}")
    # pipelined: issue PIPE calls back-to-back, block once
    PIPE = 8
    outs = [f(*dev_in) for _ in range(2)]
    jax.block_until_ready(outs)  # warm
    t0 = _time.time()
    outs = [f(*dev_in) for _ in range(PIPE)]
    jax.block_until_ready(outs)
    piped = (_time.time() - t0) / PIPE
    print(f"[bench] pipelined per-iter: {piped*1e3:.2f}ms")
    return min(min(times[1:]) if len(times) > 1 else times[0], piped)


# ---------------------------------------------------------------------------
# program builder
# ---------------------------------------------------------------------------

def build_program(hp):
    U, I, D, L = hp["U"], hp["I"], hp["D"], hp["L"]
    UT, IT = hp["UT"], hp["IT"]
    US, IS = UT * P, IT * P
    UPAD, IPAD = US * N_CORES, IS * N_CORES
    rate, rb, tr = hp["rate"], hp["rb"], hp["tr"]
    pred = hp["pred"]
    PD = hp["PD"]
    D2 = 2 * D  # packed table width (128)

    import os as _osq
    NSWQ = int(_osq.environ.get("KSWQ", "4"))
    SPKT = _osq.environ.get("KSPKT", "0") == "1"
    nc = bacc.Bacc("TRN2", target_bir_lowering=False, debug=False,
                   num_devices=N_CORES, num_swdge_queues=NSWQ)
    _qctr = [0]

    def _next_q():
        q = _qctr[0] % NSWQ
        _qctr[0] += 1
        return q

    def inp(name, shape, dt):
        return nc.dram_tensor(name, list(shape), dt, kind="ExternalInput")

    u_shard0 = inp("u_shard0", [US, D], F16)         # per-core slice
    it_shard0 = inp("it_shard0", [IS, D], F16)
    wu = inp("wu", [D, L * 4 * D], F16)   # [rate_Ws | tr_Ws | rb_Wd | tr_Wd]
    bu = inp("bu", [P, L * 4 * D], F16)
    wi = inp("wi", [D, L * 2 * D], F16)   # [rb_Ws | rate_Wd]
    bi_ = inp("bi", [P, L * 2 * D], F16)
    a_in = {g.name: inp(f"a_{g.name}", [P, L * D], F16) for g in (rate, rb, tr)}
    w1u = inp("w1u", [D, L * 2 * D], F16)  # W1 rows 0:64   [inf | int]
    w1p = inp("w1p", [D, L * 2 * D], F16)  # W1 rows 64:128 [inf | int]
    b1 = inp("b1", [P, L * 2 * D], F16)
    w2 = inp("w2", [P, L * 2 * D], F16)
    b2 = inp("b2", [P, L * 2], F32)
    iota_m_in = inp("iota_m", [P, P], F16)
    ident16_in = inp("ident16", [P, P], F16)

    g_in = {}
    for g in (rate, rb, tr):
        g_in[g.name] = {
            "idx": inp(f"{g.name}_idx", list(g.idx16[0].shape), I16),
            "idxfd": inp(f"{g.name}_idxfd", list(g.idxfd[0].shape), I16),
            "dlc": inp(f"{g.name}_dlc", list(g.dlc[0].shape), F32),
        }
    pidxu = inp("pred_idxu", list(pred.idxu[0].shape), I16)
    pidxi = inp("pred_idxi", list(pred.idxi[0].shape), I16)

    pred_out = nc.dram_tensor("pred_out", [P, pred.G_total], F32,
                              kind="ExternalOutput")
    import os
    kphase = os.environ.get("KPHASE", "full")
    dbg_spec = hp.get("dbg_spec")  # (name, rows, cols) of tensor to dump
    dbg_out = None
    if dbg_spec is not None:
        dbg_out = nc.dram_tensor("dbg_out", [dbg_spec[1], dbg_spec[2]], F32,
                                 kind="ExternalOutput")

    def internal(name, shape, shared=False, dt=F16):
        return nc.dram_tensor(name, list(shape), dt,
                              addr_space="Shared" if shared else "Local")

    u_shards = [u_shard0]
    it_shards = [it_shard0]
    fsU = {}     # l -> (agin [US,128], table [UPAD,128])
    fsI = {}     # l -> (agin [IS,128], table [IPAD,128])
    fdU = {}     # l -> local [US,128] = [fd_rb | fd_tr]
    for l in range(L):
        fsU[l] = (internal(f"agin_fsU{l}", [US, D2]),
                  internal(f"fsU{l}", [UPAD, D2]))
        fsI[l] = (internal(f"agin_fsI{l}", [IS, D2]),
                  internal(f"fsI{l}", [IPAD, D2]))
        fdU[l] = internal(f"fdU{l}", [US, D2])
        u_shards.append(internal(f"u{l + 1}", [US, D]))
        it_shards.append(internal(f"it{l + 1}", [IS, D]))
    q_sh = internal("q_sh", [US, D])
    p_sh = internal("p_sh", [US, D])
    hu_sh = internal("hu_sh", [US, PD])
    hi_sh = internal("hi_sh", [IS, PD])
    hu_t = internal("hu", [UPAD, PD])
    hi_t = internal("hi", [IPAD, PD])

    rg = [list(range(N_CORES))]

    with tile.TileContext(nc) as tc, ExitStack() as topctx:
        const = topctx.enter_context(tc.tile_pool(name="const", bufs=1))

        def cload(t, shape, dt):
            s = const.tile(list(shape), dt, tag=f"c_{t.name}")
            nc.sync.dma_start(out=s[:], in_=t.ap()[:, :])
            return s

        im = cload(iota_m_in, [P, P], F16)
        ident16 = cload(ident16_in, [P, P], F16)
        wu_sb = cload(wu, [D, L * 4 * D], F16)
        bu_sb = cload(bu, [P, L * 4 * D], F16)
        wi_sb = cload(wi, [D, L * 2 * D], F16)
        bi_sb = cload(bi_, [P, L * 2 * D], F16)
        a_sb = {nm: cload(a_in[nm], [P, L * D], F16) for nm in a_in}
        w1u_sb = cload(w1u, [D, L * 2 * D], F16)
        w1p_sb = cload(w1p, [D, L * 2 * D], F16)
        b1_sb = cload(b1, [P, L * 2 * D], F16)
        w2_sb = cload(w2, [P, L * 2 * D], F16)
        b2_sb = cload(b2, [P, L * 2], F32)

        # ------------------------------------------------------------------
        def proj_phase(l):
            """Row-sharded f16 projections; AllGather fsU/fsI tables."""
            with ExitStack() as ctx:
                sb = ctx.enter_context(tc.tile_pool(name=f"proj{l}", bufs=2))
                pst = ctx.enter_context(
                    tc.tile_pool(name=f"projt{l}", bufs=3, space="PSUM"))
                ps = ctx.enter_context(
                    tc.tile_pool(name=f"projp{l}", bufs=2, space="PSUM"))

                BT = 4

                def do(shard_tab, n_tiles, w_sb_l, b_sb_l, ncols, outs):
                    # outs: list of (dst_tensor, col_lo, col_hi)
                    for t0 in range(0, n_tiles, BT):
                        bt = min(BT, n_tiles - t0)
                        src = sb.tile([P, BT * D], F16, tag="psrc")
                        nc.sync.dma_start(
                            out=src[:, :bt * D].rearrange("p (g d) -> p g d", d=D),
                            in_=shard_tab.ap()[t0 * P:(t0 + bt) * P, :]
                            .rearrange("(g p) d -> p g d", p=P))
                        mm = ps.tile([P, BT * ncols], F32, tag="pmm",
                                     space="PSUM")
                        for k in range(bt):
                            tp = pst.tile([D, P], F16, tag="ptp", space="PSUM")
                            nc.tensor.transpose(
                                out=tp[:], in_=src[:, k * D:(k + 1) * D],
                                identity=ident16[:])
                            uT = sb.tile([D, P], F16, tag="puT")
                            nc.scalar.activation(uT[:], tp[:], Act.Copy)
                            nc.tensor.matmul(mm[:, k * ncols:(k + 1) * ncols],
                                             lhsT=uT[:], rhs=w_sb_l,
                                             start=True, stop=True)
                        big = sb.tile([P, BT * ncols], F16, tag="pbig")
                        nc.vector.tensor_tensor(
                            out=big[:, :bt * ncols]
                            .rearrange("p (g d) -> p g d", d=ncols),
                            in0=mm[:, :bt * ncols]
                            .rearrange("p (g d) -> p g d", d=ncols),
                            in1=b_sb_l.rearrange("p (g d) -> p g d", g=1)
                            .to_broadcast([P, bt, ncols]),
                            op=Alu.add)
                        for (dt_, lo, hi) in outs:
                            nc.sync.dma_start(
                                out=dt_.ap()[t0 * P:(t0 + bt) * P, :]
                                .rearrange("(g p) d -> p g d", p=P),
                                in_=big[:, :bt * ncols]
                                .rearrange("p (g d) -> p g d", d=ncols)[:, :, lo:hi])

                do(u_shards[l], UT,
                   wu_sb[:, l * 4 * D:(l + 1) * 4 * D],
                   bu_sb[:, l * 4 * D:(l + 1) * 4 * D], 4 * D,
                   [(fsU[l][0], 0, D2),        # [fs_rate | fs_tr]
                    (fdU[l], D2, 2 * D2)])     # [fd_rb | fd_tr]
                do(it_shards[l], IT,
                   wi_sb[:, l * 2 * D:(l + 1) * 2 * D],
                   bi_sb[:, l * 2 * D:(l + 1) * 2 * D], 2 * D,
                   [(fsI[l][0], 0, D2)])       # [fs_rb | fd_rate]

            import os as _os3
            if _os3.environ.get("KNOAG") == "1":
                return
            for ai, ao in (fsI[l], fsU[l]):
                nc.gpsimd.collective_compute(
                    "AllGather", Alu.bypass, replica_groups=rg,
                    ins=[ai.ap()[:, :]], outs=[ao.ap()[:, :]])

        # ------------------------------------------------------------------
        def gat_phase(l, g, fs_table, fs_lo, fd_table, fd_lo, den_first,
                      out_tensor, resid_tab):
            """Edge processing for one GAT; writes out_tensor [S, D] f16.

            fs_lo: column offset of this GAT's fs payload in the 128-col
            gathered row. den_first: ones column sits just BEFORE the fs
            payload (tr) instead of just after (rate/rb)."""
            import os as _os
            KG = int(_os.environ.get("KG", "9"))
            KFDE = _os.environ.get("KFDE", "1") == "1"
            gi = g_in[g.name]
            a_l = a_sb[g.name][:, l * D:(l + 1) * D]
            table_rows = fs_table.ap().shape[0]
            ones_col = fs_lo - 1 if den_first else fs_lo + D
            rhs_lo = fs_lo - 1 if den_first else fs_lo
            den_off = 0 if den_first else D
            num_off = 1 if den_first else 0
            with ExitStack() as ctx:
                sb = ctx.enter_context(tc.tile_pool(name=f"e{g.name}{l}", bufs=2))
                qp = ctx.enter_context(tc.tile_pool(name=f"eq{g.name}{l}", bufs=6))
                ps_acc = ctx.enter_context(
                    tc.tile_pool(name=f"ea{g.name}{l}", bufs=2, space="PSUM"))
                if KFDE:
                    ps_x = ctx.enter_context(
                        tc.tile_pool(name=f"ex{g.name}{l}", bufs=2, space="PSUM"))
                    ps_t = ctx.enter_context(
                        tc.tile_pool(name=f"et{g.name}{l}", bufs=2, space="PSUM"))

                K, Kb, nb = g.K, g.Kb, g.nb
                w_base = 0
                g_base = 0       # global sub-tile counter
                for wbi in g.blocks:
                    G = wbi * K  # sub-tiles in block
                    # loads
                    idx_t = sb.tile([P, (g.WB * K * P) // 16], I16, tag="idx")
                    c0 = g_base * P // 16
                    nc.sync.dma_start(
                        out=idx_t[:, :G * P // 16],
                        in_=gi["idx"].ap()[:, c0:c0 + G * P // 16])
                    win_of = []
                    for b in range(nb):
                        for wo in range(wbi):
                            win_of += [wo] * Kb[b]
                    if not KFDE:
                        idf_t = sb.tile([P, (g.WB * K * P) // 16], I16, tag="idf")
                        nc.sync.dma_start(
                            out=idf_t[:, :G * P // 16],
                            in_=gi["idxfd"].ap()[:, c0:c0 + G * P // 16])
                    else:
                        fd_t = sb.tile([P, g.WB * D], F16, tag="fdt")
                        nc.sync.dma_start(
                            out=fd_t[:, :wbi * D],
                            in_=fd_table.ap()[w_base * P:(w_base + wbi) * P,
                                              fd_lo:fd_lo + D]
                            .rearrange("(g p) d -> p g d", p=P))
                    dlc_t = sb.tile([P, g.WB * K], F32, tag="dlc")
                    nc.sync.dma_start(out=dlc_t[:, :G],
                                      in_=gi["dlc"].ap()[:, g_base:g_base + G])
                    if resid_tab is not None:
                        rs_t = sb.tile([P, g.WB * D], F16, tag="rs")
                        nc.sync.dma_start(
                            out=rs_t[:, :wbi * D],
                            in_=resid_tab.ap()[w_base * P:(w_base + wbi) * P, :]
                            .rearrange("(g p) d -> p g d", p=P))

                    fsg = sb.tile([P, g.WB * K * D2], F16, tag="fsg")
                    # fs gathers per bank
                    scol = 0
                    sg = 0
                    for b in range(nb):
                        ngb = wbi * Kb[b]          # sub-tiles for this bank
                        nidx = ngb * P
                        hi_row = min(table_rows, (b + 1) * BANK)
                        nc.gpsimd.dma_gather(
                            fsg[:, sg * D2:(sg + ngb) * D2]
                            .rearrange("p (g d) -> p g d", d=D2),
                            fs_table.ap()[b * BANK:hi_row, :],
                            idx_t[:, scol:scol + nidx // 16],
                            nidx, nidx, D2, single_packet=SPKT,
                            queue_num=_next_q())
                        scol += nidx // 16
                        sg += ngb
                    if not KFDE:
                        # fd gather (single range, local table)
                        fdg = sb.tile([P, g.WB * K * D2], F16, tag="fdg")
                        nc.gpsimd.dma_gather(
                            fdg[:, :G * D2].rearrange("p (g d) -> p g d", d=D2),
                            fd_table.ap()[:, :],
                            idf_t[:, :G * P // 16],
                            G * P, G * P, D2, single_packet=SPKT,
                            queue_num=_next_q())
                    # ones column for the denominator
                    nc.vector.memset(
                        fsg[:, :G * D2]
                        .rearrange("p (g d) -> p g d", d=D2)[:, :, ones_col:ones_col + 1],
                        1.0)

                    if KG <= 1:
                        out_t = sb.tile([P, g.WB * D], F16, tag="out")
                        nc.vector.tensor_tensor(
                            out=out_t[:, :wbi * D],
                            in0=fsg[:, :wbi * D],
                            in1=(fdg if not KFDE else fsg)[:, :wbi * D],
                            op=Alu.add)
                        nc.sync.dma_start(
                            out=out_tensor.ap()[w_base * P:(w_base + wbi) * P, :]
                            .rearrange("(g p) d -> p g d", p=P),
                            in_=out_t[:, :wbi * D].rearrange("p (g d) -> p g d", d=D))
                        w_base += wbi
                        g_base += G
                        continue

                    # X-stage: x = fs+fd, leaky, e = a.x, z = exp(e)
                    z_all = sb.tile([P, g.WB * K], F32, tag="zall")
                    e_all = sb.tile([P, g.WB * K], F32, tag="eall")
                    XG = 16
                    for x0 in range(0, G, XG):
                        xc = min(XG, G - x0)
                        if KFDE:
                            # x = onehot-expanded fd + fs, in PSUM via PE
                            xps = ps_x.tile([P, 16 * D], F32, tag="xps",
                                            space="PSUM")
                            for j in range(xc):
                                gg = x0 + j
                                q_t = qp.tile([P, P], F16, tag="qt")
                                nc.vector.tensor_scalar(
                                    out=q_t[:], in0=im[:],
                                    scalar1=dlc_t[:, gg:gg + 1],
                                    scalar2=None, op0=Alu.is_equal)
                                tp = ps_t.tile([P, P], F16, tag="tp",
                                               space="PSUM")
                                nc.tensor.transpose(out=tp[:], in_=q_t[:],
                                                    identity=ident16[:])
                                qtS = qp.tile([P, P], F16, tag="qtS")
                                if j % 4 != 3:
                                    nc.scalar.activation(qtS[:], tp[:], Act.Copy)
                                else:
                                    nc.vector.tensor_copy(out=qtS[:], in_=tp[:])
                                nc.tensor.matmul(
                                    xps[:, j * D:(j + 1) * D], lhsT=qtS[:],
                                    rhs=fd_t[:, win_of[gg] * D:(win_of[gg] + 1) * D],
                                    start=True, stop=False)
                                nc.tensor.matmul(
                                    xps[:, j * D:(j + 1) * D], lhsT=ident16[:],
                                    rhs=fsg[:, gg * D2 + fs_lo:gg * D2 + fs_lo + D],
                                    start=False, stop=True)
                            xl = sb.tile([P, 16 * D], F16, tag="xl")
                            nc.scalar.activation(xl[:, :xc * D],
                                                 xps[:, :xc * D], Act.Lrelu,
                                                 alpha=GAT_SLOPE)
                        else:
                            xb = sb.tile([P, 16 * D], F16, tag="xb")
                            nc.vector.tensor_tensor(
                                out=xb[:, :xc * D],
                                in0=fsg[:, x0 * D2:(x0 + xc) * D2]
                                .rearrange("p (g d) -> p g d", d=D2)[:, :, fs_lo:fs_lo + D],
                                in1=fdg[:, x0 * D2:(x0 + xc) * D2]
                                .rearrange("p (g d) -> p g d", d=D2)[:, :, fd_lo:fd_lo + D],
                                op=Alu.add)
                            xs = sb.tile([P, 16 * D], F16, tag="xs")
                            nc.vector.tensor_scalar_mul(
                                xs[:, :xc * D], xb[:, :xc * D], GAT_SLOPE)
                            xl = sb.tile([P, 16 * D], F16, tag="xl")
                            nc.vector.tensor_tensor(
                                out=xl[:, :xc * D], in0=xb[:, :xc * D],
                                in1=xs[:, :xc * D], op=Alu.max)
                        xa = sb.tile([P, 16 * D], F16, tag="xa")
                        nc.vector.tensor_tensor(
                            out=xa[:, :xc * D], in0=xl[:, :xc * D],
                            in1=a_l.rearrange("p (g d) -> p g d", g=1)
                            .to_broadcast([P, xc, D]),
                            op=Alu.mult)
                        nc.vector.reduce_sum(
                            out=e_all[:, x0:x0 + xc],
                            in_=xa[:, :xc * D].rearrange("p (g d) -> p g d", d=D),
                            axis=mybir.AxisListType.X)
                    # one Exp per block: minimizes Act func-table reloads
                    nc.scalar.activation(z_all[:, :G], e_all[:, :G], Act.Exp)

                    if KG <= 3:
                        out_t = sb.tile([P, g.WB * D], F16, tag="out")
                        nc.vector.tensor_copy(out=out_t[:, :wbi * D],
                                              in_=fsg[:, :wbi * D])
                        nc.sync.dma_start(
                            out=out_tensor.ap()[w_base * P:(w_base + wbi) * P, :]
                            .rearrange("(g p) d -> p g d", p=P),
                            in_=out_t[:, :wbi * D].rearrange("p (g d) -> p g d", d=D))
                        w_base += wbi
                        g_base += G
                        continue

                    # accumulation (window-major; PSUM groups sequential)
                    acc = ps_acc.tile([P, g.WB * (D + 1)], F32, tag="acc",
                                      space="PSUM")
                    for wo in range(wbi):
                        subs = [gg for gg in range(G) if win_of[gg] == wo]
                        for si, gg in enumerate(subs):
                            qts = qp.tile([P, P], F16, tag="qts")
                            nc.vector.tensor_scalar(
                                out=qts[:], in0=im[:],
                                scalar1=dlc_t[:, gg:gg + 1],
                                scalar2=z_all[:, gg:gg + 1],
                                op0=Alu.is_equal, op1=Alu.mult)
                            nc.tensor.matmul(
                                acc[:, wo * (D + 1):(wo + 1) * (D + 1)],
                                lhsT=qts[:],
                                rhs=fsg[:, gg * D2 + rhs_lo:gg * D2 + rhs_lo + D + 1],
                                start=(si == 0),
                                stop=(si == len(subs) - 1))

                    # divide + residual + store
                    out_t = sb.tile([P, g.WB * D], F16, tag="out")
                    den = sb.tile([P, g.WB], F32, tag="den")
                    nc.vector.tensor_scalar_max(
                        den[:, :wbi],
                        acc[:, :wbi * (D + 1)]
                        .rearrange("p (g d) -> p g d", d=D + 1)[:, :, den_off:den_off + 1],
                        1e-30)
                    rec = sb.tile([P, g.WB], F32, tag="rec")
                    nc.vector.reciprocal(rec[:, :wbi], den[:, :wbi])
                    for wo in range(wbi):
                        if resid_tab is None:
                            nc.vector.tensor_scalar_mul(
                                out_t[:, wo * D:(wo + 1) * D],
                                acc[:, wo * (D + 1) + num_off:wo * (D + 1) + num_off + D],
                                rec[:, wo:wo + 1])
                        else:
                            tmp = sb.tile([P, D], F32, tag="dtmp")
                            nc.vector.tensor_scalar_mul(
                                tmp[:], acc[:, wo * (D + 1) + num_off:wo * (D + 1) + num_off + D],
                                rec[:, wo:wo + 1])
                            nc.vector.tensor_tensor(
                                out=out_t[:, wo * D:(wo + 1) * D],
                                in0=tmp[:], in1=rs_t[:, wo * D:(wo + 1) * D],
                                op=Alu.add)
                    nc.sync.dma_start(
                        out=out_tensor.ap()[w_base * P:(w_base + wbi) * P, :]
                        .rearrange("(g p) d -> p g d", p=P),
                        in_=out_t[:, :wbi * D].rearrange("p (g d) -> p g d", d=D))

                    w_base += wbi
                    g_base += G

        # ------------------------------------------------------------------
        def epilogue_phase(l):
            with ExitStack() as ctx:
                sb = ctx.enter_context(tc.tile_pool(name=f"ep{l}", bufs=2))
                pst = ctx.enter_context(
                    tc.tile_pool(name=f"ept{l}", bufs=3, space="PSUM"))
                psm = ctx.enter_context(
                    tc.tile_pool(name=f"epm{l}", bufs=2, space="PSUM"))
                BT = 8
                w1u_l = w1u_sb[:, l * 2 * D:(l + 1) * 2 * D]
                w1p_l = w1p_sb[:, l * 2 * D:(l + 1) * 2 * D]
                b1_l = b1_sb[:, l * 2 * D:(l + 1) * 2 * D]
                w2_l = w2_sb[:, l * 2 * D:(l + 1) * 2 * D]
                b2_l = b2_sb[:, l * 2:(l + 1) * 2]
                for t0 in range(0, UT, BT):
                    bt = min(BT, UT - t0)
                    rows = slice(t0 * P, (t0 + bt) * P)
                    ut = sb.tile([P, BT * D], F16, tag="eu")
                    nc.sync.dma_start(
                        out=ut[:, :bt * D].rearrange("p (g d) -> p g d", d=D),
                        in_=u_shards[l].ap()[rows, :].rearrange("(g p) d -> p g d", p=P))
                    pt = sb.tile([P, BT * D], F16, tag="epp")
                    nc.sync.dma_start(
                        out=pt[:, :bt * D].rearrange("p (g d) -> p g d", d=D),
                        in_=p_sh.ap()[rows, :].rearrange("(g p) d -> p g d", p=P))
                    qt_ = sb.tile([P, BT * D], F16, tag="epq")
                    nc.sync.dma_start(
                        out=qt_[:, :bt * D].rearrange("p (g d) -> p g d", d=D),
                        in_=q_sh.ap()[rows, :].rearrange("(g p) d -> p g d", p=P))
                    # transposes (features on partitions) for u, p, q
                    trs = {}
                    for nm, srcp in (("u", ut), ("p", pt), ("q", qt_)):
                        big = sb.tile([D, BT * P], F16, tag=f"eT{nm}")
                        for k in range(bt):
                            tp = pst.tile([D, P], F16, tag="etp", space="PSUM")
                            nc.tensor.transpose(
                                out=tp[:], in_=srcp[:, k * D:(k + 1) * D],
                                identity=ident16[:])
                            nc.scalar.activation(big[:, k * P:(k + 1) * P],
                                                 tp[:], Act.Copy)
                        trs[nm] = big
                    # split-W1 matmuls, PSUM-accumulated: s1 = uT@W1u + (p|q)T@W1p
                    s_ps = {}
                    for ci, (nm2, col) in enumerate((("p", 0), ("q", 1))):
                        mm = psm.tile([P, BT * D], F32, tag=f"emm{ci}",
                                      space="PSUM")
                        for k in range(bt):
                            nc.tensor.matmul(
                                mm[:, k * D:(k + 1) * D],
                                lhsT=trs["u"][:, k * P:(k + 1) * P],
                                rhs=w1u_l[:, col * D:(col + 1) * D],
                                start=True, stop=False)
                            nc.tensor.matmul(
                                mm[:, k * D:(k + 1) * D],
                                lhsT=trs[nm2][:, k * P:(k + 1) * P],
                                rhs=w1p_l[:, col * D:(col + 1) * D],
                                start=False, stop=True)
                        s_ps[col] = mm
                    # batched vector tail: bias, leaky, .w2, reduce, +b2, leaky
                    s2 = []
                    for col in (0, 1):
                        s1 = sb.tile([P, BT * D], F16, tag=f"es1_{col}")
                        nc.vector.tensor_tensor(
                            out=s1[:, :bt * D].rearrange("p (g d) -> p g d", d=D),
                            in0=s_ps[col][:, :bt * D].rearrange("p (g d) -> p g d", d=D),
                            in1=b1_l[:, col * D:(col + 1) * D]
                            .rearrange("p (g d) -> p g d", g=1)
                            .to_broadcast([P, bt, D]),
                            op=Alu.add)
                        s1s = sb.tile([P, BT * D], F16, tag=f"es1s_{col}")
                        nc.vector.tensor_scalar_mul(
                            s1s[:, :bt * D], s1[:, :bt * D], MLP_SLOPE)
                        s1l = sb.tile([P, BT * D], F16, tag=f"es1l_{col}")
                        nc.vector.tensor_tensor(
                            out=s1l[:, :bt * D], in0=s1[:, :bt * D],
                            in1=s1s[:, :bt * D], op=Alu.max)
                        xw = sb.tile([P, BT * D], F16, tag=f"exw_{col}")
                        nc.vector.tensor_tensor(
                            out=xw[:, :bt * D].rearrange("p (g d) -> p g d", d=D),
                            in0=s1l[:, :bt * D].rearrange("p (g d) -> p g d", d=D),
                            in1=w2_l[:, col * D:(col + 1) * D]
                            .rearrange("p (g d) -> p g d", g=1)
                            .to_broadcast([P, bt, D]),
                            op=Alu.mult)
                        sv = sb.tile([P, BT], F32, tag=f"esv_{col}")
                        nc.vector.reduce_sum(
                            out=sv[:, :bt],
                            in_=xw[:, :bt * D].rearrange("p (g d) -> p g d", d=D),
                            axis=mybir.AxisListType.X)
                        svb = sb.tile([P, BT], F32, tag=f"esvb_{col}")
                        nc.vector.tensor_scalar_add(
                            svb[:, :bt], sv[:, :bt], b2_l[:, col:col + 1])
                        svs = sb.tile([P, BT], F32, tag=f"esvs_{col}")
                        nc.vector.tensor_scalar_mul(
                            svs[:, :bt], svb[:, :bt], MLP_SLOPE)
                        svl = sb.tile([P, BT], F32, tag=f"esvl_{col}")
                        nc.vector.tensor_tensor(
                            out=svl[:, :bt], in0=svb[:, :bt],
                            in1=svs[:, :bt], op=Alu.max)
                        s2.append(svl)
                    dg = sb.tile([P, BT], F32, tag="edg")
                    nc.vector.tensor_tensor(
                        out=dg[:, :bt], in0=s2[0][:, :bt], in1=s2[1][:, :bt],
                        op=Alu.subtract)
                    g0 = sb.tile([P, BT], F16, tag="eg0")
                    nc.scalar.activation(g0[:, :bt], dg[:, :bt], Act.Sigmoid)
                    # out = u + q + g0*(p - q)
                    pq = sb.tile([P, BT * D], F16, tag="epq2")
                    nc.vector.tensor_tensor(
                        out=pq[:, :bt * D], in0=pt[:, :bt * D],
                        in1=qt_[:, :bt * D], op=Alu.subtract)
                    gpq = sb.tile([P, BT * D], F16, tag="egpq")
                    nc.vector.tensor_tensor(
                        out=gpq[:, :bt * D].rearrange("p (g d) -> p g d", d=D),
                        in0=pq[:, :bt * D].rearrange("p (g d) -> p g d", d=D),
                        in1=g0[:, :bt].rearrange("p (g d) -> p g d", d=1)
                        .to_broadcast([P, bt, D]),
                        op=Alu.mult)
                    uq = sb.tile([P, BT * D], F16, tag="euq")
                    nc.vector.tensor_tensor(
                        out=uq[:, :bt * D], in0=ut[:, :bt * D],
                        in1=qt_[:, :bt * D], op=Alu.add)
                    ot = sb.tile([P, BT * D], F16, tag="eo")
                    nc.vector.tensor_tensor(
                        out=ot[:, :bt * D], in0=uq[:, :bt * D],
                        in1=gpq[:, :bt * D], op=Alu.add)
                    nc.sync.dma_start(
                        out=u_shards[l + 1].ap()[rows, :]
                        .rearrange("(g p) d -> p g d", p=P),
                        in_=ot[:, :bt * D].rearrange("p (g d) -> p g d", d=D))

        # ------------------------------------------------------------------
        def hu_build_phase(which):
            """Assemble hu_sh [US, PD] / hi_sh f16 locally, then AllGather."""
            with ExitStack() as ctx:
                sb = ctx.enter_context(tc.tile_pool(name=f"hub{which}", bufs=2))
                BT = 16
                for shards, out_tab, n_tiles in (((u_shards, hu_sh, UT),)
                                                 if which == "u" else
                                                 ((it_shards, hi_sh, IT),)):
                    for t0 in range(0, n_tiles, BT):
                        bt = min(BT, n_tiles - t0)
                        rows = slice(t0 * P, (t0 + bt) * P)
                        big = sb.tile([P, BT * PD], F16, tag="hbig")
                        nc.vector.memset(
                            big[:, :bt * PD]
                            .rearrange("p (g d) -> p g d", d=PD)
                            [:, :, (L + 1) * D:PD], 0)
                        for li, tab in enumerate(shards):
                            ld = sb.tile([P, BT * D], F16, tag="hld")
                            nc.sync.dma_start(
                                out=ld[:, :bt * D].rearrange("p (g d) -> p g d", d=D),
                                in_=tab.ap()[rows, :]
                                .rearrange("(g p) d -> p g d", p=P))
                            nc.vector.tensor_copy(
                                out=big[:, :bt * PD]
                                .rearrange("p (g d) -> p g d", d=PD)
                                [:, :, li * D:(li + 1) * D],
                                in_=ld[:, :bt * D]
                                .rearrange("p (g d) -> p g d", d=D))
                        nc.sync.dma_start(
                            out=out_tab.ap()[rows, :]
                            .rearrange("(g p) d -> p g d", p=P),
                            in_=big[:, :bt * PD].rearrange("p (g d) -> p g d", d=PD))
            import os as _os4
            if _os4.environ.get("KNOAG") == "1":
                return
            ai, ao = (hu_sh, hu_t) if which == "u" else (hi_sh, hi_t)
            nc.gpsimd.collective_compute(
                "AllGather", Alu.bypass, replica_groups=rg,
                ins=[ai.ap()[:, :]], outs=[ao.ap()[:, :]])

        # ------------------------------------------------------------------
        def pred_phase():
            with ExitStack() as ctx:
                sb = ctx.enter_context(tc.tile_pool(name="pred", bufs=2))
                G = pred.G_blk
                for bi in range(pred.n_blocks):
                    hu_g = sb.tile([P, G * PD], F16, tag="phu")
                    hi_g = sb.tile([P, G * PD], F16, tag="phi")
                    iu_t = sb.tile([P, G * P // 16], I16, tag="piu")
                    c0 = bi * G * P // 16
                    nc.sync.dma_start(out=iu_t[:],
                                      in_=pidxu.ap()[:, c0:c0 + G * P // 16])
                    ii_t = sb.tile([P, G * P // 16], I16, tag="pii")
                    nc.sync.dma_start(out=ii_t[:],
                                      in_=pidxi.ap()[:, c0:c0 + G * P // 16])
                    # hi gathers first: hi_t is AllGathered early, so these
                    # overlap the trailing hu AllGather on the in-order queue
                    sg = 0
                    scol = 0
                    for u_ in range(pred.nbu):
                        for i_ in range(pred.nbi):
                            ngb = pred.Kp[(u_, i_)]
                            nidx = ngb * P
                            hi_row = min(hi_t.ap().shape[0], (i_ + 1) * BANK)
                            nc.gpsimd.dma_gather(
                                hi_g[:, sg * PD:(sg + ngb) * PD]
                                .rearrange("p (g d) -> p g d", d=PD),
                                hi_t.ap()[i_ * BANK:hi_row, :],
                                ii_t[:, scol:scol + nidx // 16],
                                nidx, nidx, PD, single_packet=SPKT,
                                queue_num=_next_q())
                            sg += ngb
                            scol += nidx // 16
                    # hu gathers: per user bank (spans its item-bank pairs)
                    sg = 0
                    scol = 0
                    for u_ in range(pred.nbu):
                        ngb = sum(pred.Kp[(u_, i_)] for i_ in range(pred.nbi))
                        nidx = ngb * P
                        hi_row = min(hu_t.ap().shape[0], (u_ + 1) * BANK)
                        nc.gpsimd.dma_gather(
                            hu_g[:, sg * PD:(sg + ngb) * PD]
                            .rearrange("p (g d) -> p g d", d=PD),
                            hu_t.ap()[u_ * BANK:hi_row, :],
                            iu_t[:, scol:scol + nidx // 16],
                            nidx, nidx, PD, single_packet=SPKT,
                            queue_num=_next_q())
                        sg += ngb
                        scol += nidx // 16
                    # dots (batched f16 mult + per-group reduce)
                    dt_ = sb.tile([P, G], F32, tag="pdot")
                    for x0 in range(0, G, 8):
                        xc = min(8, G - x0)
                        prod = sb.tile([P, 8 * PD], F16, tag="pprod")
                        nc.vector.tensor_tensor(
                            out=prod[:, :xc * PD],
                            in0=hu_g[:, x0 * PD:(x0 + xc) * PD],
                            in1=hi_g[:, x0 * PD:(x0 + xc) * PD], op=Alu.mult)
                        nc.vector.reduce_sum(
                            out=dt_[:, x0:x0 + xc],
                            in_=prod[:, :xc * PD]
                            .rearrange("p (g d) -> p g d", d=PD),
                            axis=mybir.AxisListType.X)
                    nc.sync.dma_start(out=pred_out.ap()[:, bi * G:(bi + 1) * G],
                                      in_=dt_[:])

        # ------------------------------------------------------------------
        phase_order = []
        for l in range(L):
            phase_order += [f"proj{l}", f"rb{l}", f"rate{l}"]
            if l == L - 1:
                phase_order += ["hib"]
            phase_order += [f"tr{l}", f"epi{l}"]
        phase_order += ["hu", "pred"]

        global PHASE_MARKS
        PHASE_MARKS = []

        def run_until():
            for ph in phase_order:
                PHASE_MARKS.append((ph, nc.next_id()))
                l = int(ph[-1]) if ph[-1].isdigit() else 0
                if ph.startswith("proj"):
                    proj_phase(l)
                elif ph.startswith("rate"):
                    # fs from fsU cols 0:64, fd from fsI-agin cols 64:128
                    gat_phase(l, rate, fsU[l][1], 0, fsI[l][0], D, False,
                              it_shards[l + 1], it_shards[l])
                elif ph.startswith("rb"):
                    # fs from fsI cols 0:64, fd from fdU cols 0:64
                    gat_phase(l, rb, fsI[l][1], 0, fdU[l], 0, False,
                              q_sh, None)
                elif ph.startswith("tr"):
                    # fs from fsU cols 64:128, fd from fdU cols 64:128
                    gat_phase(l, tr, fsU[l][1], D, fdU[l], D, True,
                              p_sh, None)
                elif ph.startswith("epi"):
                    epilogue_phase(l)
                elif ph == "hib":
                    hu_build_phase("i")
                elif ph == "hu":
                    hu_build_phase("u")
                elif ph == "pred":
                    pred_phase()
                if ph == kphase:
                    return

        run_until()
        if dbg_out is not None:
            dbg_tensors = dict(
                q_sh=q_sh, p_sh=p_sh, hu=hu_t, hi=hi_t, hu_sh=hu_sh,
                hi_sh=hi_sh,
                **{f"u_shard{i}": t for i, t in enumerate(u_shards)},
                **{f"it_shard{i}": t for i, t in enumerate(it_shards)},
                **{f"fsU{l}": fsU[l][1] for l in range(L)},
                **{f"fsI{l}": fsI[l][1] for l in range(L)},
                **{f"agin_fsU{l}": fsU[l][0] for l in range(L)},
                **{f"agin_fsI{l}": fsI[l][0] for l in range(L)},
                **{f"fdU{l}": fdU[l] for l in range(L)},
            )
            src_t = dbg_tensors[dbg_spec[0]]
            sdt = src_t.ap().dtype
            with ExitStack() as ctx:
                sbd = ctx.enter_context(tc.tile_pool(name="dbg", bufs=2))
                rows, cols = dbg_spec[1], dbg_spec[2]
                for r0 in range(0, rows, P):
                    rc = min(P, rows - r0)
                    t_ = sbd.tile([P, cols], sdt, tag="dbg")
                    nc.sync.dma_start(out=t_[:rc, :],
                                      in_=src_t.ap()[r0:r0 + rc, :])
                    if sdt != F32:
                        t2 = sbd.tile([P, cols], F32, tag="dbg2")
                        nc.vector.tensor_copy(out=t2[:rc, :], in_=t_[:rc, :])
                        t_ = t2
                    nc.sync.dma_start(out=dbg_out.ap()[r0:r0 + rc, :],
                                      in_=t_[:rc, :])

    nc.compile()
    return nc


# ---------------------------------------------------------------------------
# entry point
# ---------------------------------------------------------------------------

def _pad_rows(a, rows):
    out = np.zeros((rows, a.shape[1]), dtype=a.dtype)
    out[:a.shape[0]] = a
    return out


def kernel(**inputs):
    U, D = inputs["user_emb"].shape
    I = inputs["item_emb"].shape[0]
    L = inputs["rate_Ws"].shape[0]
    UT = _ceil(_ceil(U, P), N_CORES)
    IT = _ceil(_ceil(I, P), N_CORES)
    US, IS = UT * P, IT * P
    UPAD, IPAD = US * N_CORES, IS * N_CORES
    # gather elem size must be a multiple of 256 bytes -> PD*2 % 256 == 0
    PD = _ceil(D * (L + 1) * 2, 256) * 128

    rate_src = np.asarray(inputs["rate_src"])
    rate_dst = np.asarray(inputs["rate_dst"])
    trust_src = np.asarray(inputs["trust_src"])
    trust_dst = np.asarray(inputs["trust_dst"])

    rate = GatStruct("rate", rate_src, rate_dst, UPAD, IT)
    rb = GatStruct("rb", rate_dst, rate_src, IPAD, UT)
    tr = GatStruct("tr", trust_src, trust_dst, UPAD, UT)

    pos_src = np.asarray(inputs["pos_src"])
    pos_dst = np.asarray(inputs["pos_dst"])
    neg_src = np.asarray(inputs["neg_src"])
    neg_dst = np.asarray(inputs["neg_dst"])
    psrc = np.concatenate([pos_src, neg_src])
    pdst = np.concatenate([pos_dst, neg_dst])
    pred = PredStruct(psrc, pdst, UPAD, IPAD, block_edges=6144)

    import os
    hp = dict(U=U, I=I, D=D, L=L, UT=UT, IT=IT, PD=PD,
              rate=rate, rb=rb, tr=tr, pred=pred)
    print(f"[kernel] struct: rate K={rate.K} Kb={rate.Kb} WB={rate.WB} blocks={len(rate.blocks)}; "
          f"rb K={rb.K} WB={rb.WB} blocks={len(rb.blocks)}; "
          f"tr K={tr.K} WB={tr.WB} blocks={len(tr.blocks)}; "
          f"pred G_blk={pred.G_blk} blocks={pred.n_blocks}")
    kdbg = os.environ.get("KDBG")
    if kdbg:
        shp = {}
        for i in range(L + 1):
            shp[f"u_shard{i}"] = (US, D); shp[f"it_shard{i}"] = (IS, D)
        for l in range(L):
            shp[f"fsU{l}"] = (UPAD, 2 * D); shp[f"fsI{l}"] = (IPAD, 2 * D)
            shp[f"agin_fsU{l}"] = (US, 2 * D); shp[f"agin_fsI{l}"] = (IS, 2 * D)
            shp[f"fdU{l}"] = (US, 2 * D)
        shp["q_sh"] = (US, D); shp["p_sh"] = (US, D)
        shp["hu"] = (UPAD, PD); shp["hi"] = (IPAD, PD)
        shp["hu_sh"] = (US, PD); shp["hi_sh"] = (IS, PD)
        hp["dbg_spec"] = (kdbg, *shp[kdbg])

    t_b = __import__("time").time()
    nc = build_program(hp)
    print(f"[kernel] build+compile: {__import__('time').time() - t_b:.1f}s")

    # ---- inputs ----
    f16 = NPF16
    ue_pad = _pad_rows(inputs["user_emb"], UPAD).astype(f16)
    ie_pad = _pad_rows(inputs["item_emb"], IPAD).astype(f16)
    wu = np.concatenate([
        np.concatenate([inputs["rate_Ws"][l], inputs["tr_Ws"][l],
                        inputs["rb_Wd"][l], inputs["tr_Wd"][l]], axis=1)
        for l in range(L)], axis=1).astype(f16)
    bu = np.concatenate([
        np.tile(np.concatenate([inputs["rate_bs"][l], inputs["tr_bs"][l],
                                inputs["rb_bd"][l], inputs["tr_bd"][l]])[None, :],
                (P, 1))
        for l in range(L)], axis=1).astype(f16)
    wi = np.concatenate([
        np.concatenate([inputs["rb_Ws"][l], inputs["rate_Wd"][l]], axis=1)
        for l in range(L)], axis=1).astype(f16)
    bi_ = np.concatenate([
        np.tile(np.concatenate([inputs["rb_bs"][l], inputs["rate_bd"][l]])[None, :],
                (P, 1))
        for l in range(L)], axis=1).astype(f16)
    a_arrs = {}
    for nm in ("rate", "rb", "tr"):
        a_arrs[nm] = np.concatenate([
            np.tile(np.asarray(inputs[f"{nm}_a"][l])[None, :], (P, 1))
            for l in range(L)], axis=1).astype(f16)
    w1u = np.concatenate([
        np.concatenate([inputs["inf_W1"][l][:D], inputs["int_W1"][l][:D]],
                       axis=1)
        for l in range(L)], axis=1).astype(f16)
    w1p = np.concatenate([
        np.concatenate([inputs["inf_W1"][l][D:], inputs["int_W1"][l][D:]],
                       axis=1)
        for l in range(L)], axis=1).astype(f16)
    b1 = np.concatenate([
        np.tile(np.concatenate([inputs["inf_b1"][l], inputs["int_b1"][l]])[None, :],
                (P, 1))
        for l in range(L)], axis=1).astype(f16)
    w2 = np.concatenate([
        np.tile(np.concatenate([inputs["inf_W2"][l][:, 0],
                                inputs["int_W2"][l][:, 0]])[None, :], (P, 1))
        for l in range(L)], axis=1).astype(f16)
    b2 = np.concatenate([
        np.tile(np.array([[inputs["inf_b2"][l][0], inputs["int_b2"][l][0]]],
                         dtype=np.float32), (P, 1))
        for l in range(L)], axis=1).astype(np.float32)
    iota = np.arange(P, dtype=np.float32)
    iota_m = np.tile(iota[None, :], (P, 1)).astype(f16)
    ident16 = np.eye(P, dtype=f16)

    in_maps = []
    for c in range(N_CORES):
        m = {
            "u_shard0": ue_pad[c * US:(c + 1) * US],
            "it_shard0": ie_pad[c * IS:(c + 1) * IS],
            "wu": wu, "bu": bu, "wi": wi, "bi": bi_,
            "a_rate": a_arrs["rate"], "a_rb": a_arrs["rb"], "a_tr": a_arrs["tr"],
            "w1u": w1u, "w1p": w1p, "b1": b1, "w2": w2, "b2": b2,
            "iota_m": iota_m, "ident16": ident16,
            "pred_idxu": pred.idxu[c], "pred_idxi": pred.idxi[c],
        }
        for g in (rate, rb, tr):
            m[f"{g.name}_idx"] = g.idx16[c]
            m[f"{g.name}_idxfd"] = g.idxfd[c]
            m[f"{g.name}_dlc"] = g.dlc[c]
        in_maps.append(m)

    trace = os.environ.get("KTRACE") == "1"
    t_run = __import__("time").time()
    res = run_bass_kernel_spmd(nc, in_maps, core_ids=list(range(N_CORES)),
                               trace=trace)
    print(f"[kernel] device run wall: {__import__('time').time() - t_run:.1f}s")
    global LAST_RES, LAST_HP, LAST_EXEC_NS
    LAST_RES, LAST_HP, LAST_EXEC_NS = res, hp, res.exec_time_ns
    if os.environ.get("KBENCH") == "1":
        tmin = bench_pjrt(nc, in_maps, iters=int(os.environ.get("KBENCH_ITERS", "4")))
        LAST_EXEC_NS = int(tmin * 1e9)

    # ---- assemble outputs ----
    E = len(psrc)
    out = np.zeros((E,), dtype=np.float32)
    for c in range(N_CORES):
        vals = res.results[c]["pred_out"]  # [128, G_total]
        smap = pred.slotmap[c]
        gidx = np.arange(len(smap))
        v = vals[gidx % P, gidx // P]
        ok = smap >= 0
        out[smap[ok]] = v[ok]
    pos = out[:len(pos_src)].reshape(-1, 1)
    neg = out[len(pos_src):].reshape(-1, 1)
    return pos, neg


# revision 33
# speedup vs baseline: 1.0303x; 1.0303x over previous
"""DiffNet++ (GATv2 diffusion + gamma gating + dot-product prediction) on 8
Trainium2 NeuronCores via Bass/Tile.  v2 — DVE-light edge pipeline.

Strategy (dst-range edge sharding, one SPMD program):
  - Users/items row-sharded: users 98 tiles (12544 rows)/core, items 49 tiles
    (6272 rows)/core. Each GAT edge belongs to the core owning its dst.
  - Projections in f16, packed into 128-col tables so dma_gather rows are
    exactly 256B: fsU = [fs_rate | fs_tr] (AllGathered, UPAD rows),
    fsI = [fs_rb | fd_rate] (AllGathered, IPAD rows), fdU = [fd_rb | fd_tr]
    (local US rows).
  - Per edge slot, gather BOTH fs[src] (banked, from the global table) and
    fd[dst] (single-range, from the local table). x = fs+fd, leaky, e = a.x,
    z = exp(e) — batched f16 vector ops, exp on the scalar engine.
  - Segment softmax without max subtraction (logits ~1e-2): out[v] =
    (sum_e z_e fs[src]) / (sum_e z_e) via one z-scaled one-hot matmul per
    sub-tile: onehot = (iota == dlc) * z built in ONE 4x tensor_scalar op;
    a 1.0 column memset into the gathered fs tile makes the denominator a
    free 65th matmul column.
  - Epilogue (gamma gating MLPs) batched: per-tile f16 transposes + split-W1
    PSUM-accumulated matmuls, vector work batched over 8 node tiles.
  - hu/hi concat tables built locally in f16 [*, 256] and AllGathered once.
  - Prediction: gather both sides per edge (512B rows); fused f16 dots.
"""
import sys

sys.path.insert(0, "/opt/trn_rl_repo")

from contextlib import ExitStack

import numpy as np

import concourse.bass as bass
import concourse.tile as tile
from concourse import bacc, mybir
from concourse.bass_utils import run_bass_kernel_spmd

N_CORES = 8
P = 128
BANK = 32768
GAT_SLOPE = 0.2
MLP_SLOPE = 0.01
F16 = mybir.dt.float16
F32 = mybir.dt.float32
I16 = mybir.dt.int16
NPF16 = np.dtype("float16")

Alu = mybir.AluOpType
Act = mybir.ActivationFunctionType


def _ceil(a, b):
    return -(-a // b)


# ---------------------------------------------------------------------------
# host-side preprocessing
# ---------------------------------------------------------------------------

class GatStruct:
    """Canonical (core-uniform) structure for one GAT graph's edges."""

    def __init__(self, name, src, dst, table_rows, shard_tiles):
        self.name = name
        self.nb = _ceil(table_rows, BANK)
        self.shard_tiles = shard_tiles
        S = shard_tiles * P
        self.S = S

        core = np.minimum(dst // S, N_CORES - 1)
        win = (dst - core * S) // P
        bank = src // BANK

        cnt = np.zeros((N_CORES, shard_tiles, self.nb), dtype=np.int64)
        np.add.at(cnt, (core, win, bank), 1)
        self.Kb = [max(1, int(_ceil(int(cnt[:, :, b].max()), P)))
                   for b in range(self.nb)]
        self.K = sum(self.Kb)
        self.WB = max(1, min(7, 144 // self.K))
        self.blocks = []
        t = shard_tiles
        while t > 0:
            wbi = min(self.WB, t)
            self.blocks.append(wbi)
            t -= wbi
        self.G_total = shard_tiles * self.K  # sub-tiles per core overall
        self.total_cols = self.G_total * P // 16

        order = np.lexsort((src, bank, win, core))
        src_s = src[order]
        dst_s = dst[order]
        core_s = core[order]
        win_s = win[order]
        bank_s = bank[order]

        self.idx16 = []    # fs gather: src - bank*BANK
        self.idxfd = []    # fd gather: dst - core*S (local row)
        self.dlc = []      # dst-local-in-window (-1 pad), [128, G_total] f32
        for c in range(N_CORES):
            sel = core_s == c
            csrc = src_s[sel]
            cdst = dst_s[sel]
            cwin = win_s[sel]
            cbank = bank_s[sel]
            key = cwin.astype(np.int64) * self.nb + cbank
            ids = np.zeros((self.G_total * P,), dtype=np.int16)
            idf = np.zeros((self.G_total * P,), dtype=np.int16)
            dl = np.full((self.G_total * P,), -1.0, dtype=np.float32)
            # slot layout: per block: [bank b: [window wo: Kb[b]*128 slots]]
            slot0 = 0
            w_base = 0
            for wbi in self.blocks:
                for b in range(self.nb):
                    for wo in range(wbi):
                        w = w_base + wo
                        e0 = np.searchsorted(key, w * self.nb + b, "left")
                        e1 = np.searchsorted(key, w * self.nb + b, "right")
                        n = e1 - e0
                        nsw = self.Kb[b] * P
                        assert n <= nsw, (name, c, w, b, n, nsw)
                        ids[slot0:slot0 + n] = (csrc[e0:e1] - b * BANK).astype(np.int16)
                        idf[slot0:slot0 + n] = (cdst[e0:e1] - c * S).astype(np.int16)
                        dl[slot0:slot0 + n] = (cdst[e0:e1] - (c * S + w * P)).astype(np.float32)
                        slot0 += nsw
                w_base += wbi
            assert slot0 == self.G_total * P
            cols = self.total_cols
            j = np.arange(self.G_total * P)

            def wrap(v):
                a = np.empty((16, cols), dtype=np.int16)
                a[j % 16, j // 16] = v
                return np.tile(a, (8, 1))

            self.idx16.append(wrap(ids))
            self.idxfd.append(wrap(idf))
            self.dlc.append(np.ascontiguousarray(
                dl.reshape(self.G_total, P).T))          # [128, G_total]


class PredStruct:
    """Canonical structure for prediction edges (pos+neg concatenated)."""

    def __init__(self, src, dst, u_rows, i_rows, block_edges):
        E = len(src)
        assert E % N_CORES == 0
        per_core = E // N_CORES
        self.per_core = per_core
        self.nbu = _ceil(u_rows, BANK)
        self.nbi = _ceil(i_rows, BANK)
        self.n_blocks = _ceil(per_core, block_edges)
        pairs = [(u_, i_) for u_ in range(self.nbu) for i_ in range(self.nbi)]
        self.pairs = pairs

        core = np.arange(E) // per_core
        blk = (np.arange(E) % per_core) // block_edges
        ub = src // BANK
        ib = dst // BANK
        cnt = np.zeros((N_CORES, self.n_blocks, self.nbu, self.nbi), dtype=np.int64)
        np.add.at(cnt, (core, blk, ub, ib), 1)
        self.Kp = {pq: max(1, int(_ceil(int(cnt[:, :, pq[0], pq[1]].max()), P)))
                   for pq in pairs}
        self.G_blk = sum(self.Kp[pq] for pq in pairs)
        self.G_total = self.G_blk * self.n_blocks

        self.idxu = []
        self.idxi = []
        self.slotmap = []
        for c in range(N_CORES):
            lo = c * per_core
            cs = src[lo:lo + per_core]
            cd = dst[lo:lo + per_core]
            iu = np.zeros((self.G_total * P,), dtype=np.int16)
            ii = np.zeros((self.G_total * P,), dtype=np.int16)
            smap = np.full((self.G_total * P,), -1, dtype=np.int64)
            for bi in range(self.n_blocks):
                b0 = bi * block_edges
                b1 = min(b0 + block_edges, per_core)
                bs, bd = cs[b0:b1], cd[b0:b1]
                bub, bib = bs // BANK, bd // BANK
                key = bub.astype(np.int64) * self.nbi + bib
                ordk = np.lexsort((bs, key))
                keys = key[ordk]
                off = bi * self.G_blk * P
                for pq in pairs:
                    kv = pq[0] * self.nbi + pq[1]
                    e0 = np.searchsorted(keys, kv, "left")
                    e1 = np.searchsorted(keys, kv, "right")
                    n = e1 - e0
                    npad = self.Kp[pq] * P
                    assert n <= npad
                    sel2 = ordk[e0:e1]
                    iu[off:off + n] = (bs[sel2] - pq[0] * BANK).astype(np.int16)
                    ii[off:off + n] = (bd[sel2] - pq[1] * BANK).astype(np.int16)
                    smap[off:off + n] = lo + b0 + sel2
                    off += npad
            cols = self.G_total * P // 16
            j = np.arange(self.G_total * P)
            au = np.empty((16, cols), dtype=np.int16)
            au[j % 16, j // 16] = iu
            ai = np.empty((16, cols), dtype=np.int16)
            ai[j % 16, j // 16] = ii
            self.idxu.append(np.tile(au, (8, 1)))
            self.idxi.append(np.tile(ai, (8, 1)))
            self.slotmap.append(smap)


# ---------------------------------------------------------------------------
# benchmarking helper (steady-state pipelined timing via PJRT)
# ---------------------------------------------------------------------------

def bench_pjrt(nc, in_maps, iters=3):
    """Time steady-state executions of the compiled program on the 8 cores."""
    import time as _time
    import jax
    from jax.sharding import Mesh, PartitionSpec
    from jax.experimental.shard_map import shard_map
    from concourse import bass2jax
    from concourse import mybir as _mb

    bass2jax.install_neuronx_cc_hook()
    partition_name = (nc.partition_id_tensor.name
                      if nc.partition_id_tensor else None)
    in_names, out_names, out_avals = [], [], []
    for alloc in nc.m.functions[0].allocations:
        if not isinstance(alloc, _mb.MemoryLocationSet):
            continue
        name = alloc.memorylocations[0].name
        if alloc.kind == "ExternalInput":
            if name != partition_name:
                in_names.append(name)
        elif alloc.kind == "ExternalOutput":
            out_names.append(name)
            out_avals.append(jax.core.ShapedArray(
                tuple(alloc.tensor_shape), _mb.dt.np(alloc.dtype)))
    n_params = len(in_names)
    zero_outs = [np.zeros(a.shape, a.dtype) for a in out_avals]
    all_names = in_names + out_names
    if partition_name is not None:
        all_names = all_names + [partition_name]

    def _body(*args):
        operands = list(args)
        if partition_name is not None:
            operands.append(bass2jax.partition_id_tensor())
        return tuple(bass2jax._bass_exec_p.bind(
            *operands, out_avals=tuple(out_avals),
            in_names=tuple(all_names), out_names=tuple(out_names),
            lowering_input_output_aliases=(), sim_require_finite=True,
            sim_require_nnan=True, nc=nc))

    devices = jax.devices()[:N_CORES]
    mesh = Mesh(np.asarray(devices), ("core",))
    nspec = n_params + len(out_names)
    f = jax.jit(shard_map(_body, mesh=mesh,
                          in_specs=(PartitionSpec("core"),) * nspec,
                          out_specs=(PartitionSpec("core"),) * len(out_names),
                          check_rep=False), keep_unused=True)
    from jax.sharding import NamedSharding
    sh = NamedSharding(mesh, PartitionSpec("core"))
    concat_in = [np.concatenate([np.asarray(m[nm]) for m in in_maps], axis=0)
                 for nm in in_names]
    concat_in += [np.concatenate([z] * N_CORES, axis=0) for z in zero_outs]
    dev_in = [jax.device_put(x, sh) for x in concat_in]
    times = []
    for i in range(iters):
        t0 = _time.time()
        outs = f(*dev_in)
        jax.block_until_ready(outs)
        times.append(_time.time() - t0)
    print(f"[bench] iter times: {[f'{t*1e3:.2f}ms' for t in times]}")
    # pipelined: issue PIPE calls back-to-back, block once
    PIPE = 8
    outs = [f(*dev_in) for _ in range(2)]
    jax.block_until_ready(outs)  # warm
    t0 = _time.time()
    outs = [f(*dev_in) for _ in range(PIPE)]
    jax.block_until_ready(outs)
    piped = (_time.time() - t0) / PIPE
    print(f"[bench] pipelined per-iter: {piped*1e3:.2f}ms")
    return min(min(times[1:]) if len(times) > 1 else times[0], piped)


# ---------------------------------------------------------------------------
# program builder
# ---------------------------------------------------------------------------

def build_program(hp):
    U, I, D, L = hp["U"], hp["I"], hp["D"], hp["L"]
    UT, IT = hp["UT"], hp["IT"]
    US, IS = UT * P, IT * P
    UPAD, IPAD = US * N_CORES, IS * N_CORES
    rate, rb, tr = hp["rate"], hp["rb"], hp["tr"]
    pred = hp["pred"]
    PD = hp["PD"]
    D2 = 2 * D  # packed table width (128)

    import os as _osq
    NSWQ = int(_osq.environ.get("KSWQ", "4"))
    SPKT = _osq.environ.get("KSPKT", "0") == "1"
    nc = bacc.Bacc("TRN2", target_bir_lowering=False, debug=False,
                   num_devices=N_CORES, num_swdge_queues=NSWQ)
    _qctr = [0]

    def _next_q():
        q = _qctr[0] % NSWQ
        _qctr[0] += 1
        return q

    def inp(name, shape, dt):
        return nc.dram_tensor(name, list(shape), dt, kind="ExternalInput")

    u_shard0 = inp("u_shard0", [US, D], F16)         # per-core slice
    it_shard0 = inp("it_shard0", [IS, D], F16)
    wu = inp("wu", [D, L * 4 * D], F16)   # [rate_Ws | tr_Ws | rb_Wd | tr_Wd]
    bu = inp("bu", [P, L * 4 * D], F16)
    wi = inp("wi", [D, L * 2 * D], F16)   # [rb_Ws | rate_Wd]
    bi_ = inp("bi", [P, L * 2 * D], F16)
    a_in = {g.name: inp(f"a_{g.name}", [P, L * D], F16) for g in (rate, rb, tr)}
    w1u = inp("w1u", [D, L * 2 * D], F16)  # W1 rows 0:64   [inf | int]
    w1p = inp("w1p", [D, L * 2 * D], F16)  # W1 rows 64:128 [inf | int]
    b1 = inp("b1", [P, L * 2 * D], F16)
    w2 = inp("w2", [P, L * 2 * D], F16)
    b2 = inp("b2", [P, L * 2], F32)
    iota_m_in = inp("iota_m", [P, P], F16)
    ident16_in = inp("ident16", [P, P], F16)

    g_in = {}
    for g in (rate, rb, tr):
        g_in[g.name] = {
            "idx": inp(f"{g.name}_idx", list(g.idx16[0].shape), I16),
            "idxfd": inp(f"{g.name}_idxfd", list(g.idxfd[0].shape), I16),
            "dlc": inp(f"{g.name}_dlc", list(g.dlc[0].shape), F32),
        }
    pidxu = inp("pred_idxu", list(pred.idxu[0].shape), I16)
    pidxi = inp("pred_idxi", list(pred.idxi[0].shape), I16)

    pred_out = nc.dram_tensor("pred_out", [P, pred.G_total], F32,
                              kind="ExternalOutput")
    import os
    kphase = os.environ.get("KPHASE", "full")
    dbg_spec = hp.get("dbg_spec")  # (name, rows, cols) of tensor to dump
    dbg_out = None
    if dbg_spec is not None:
        dbg_out = nc.dram_tensor("dbg_out", [dbg_spec[1], dbg_spec[2]], F32,
                                 kind="ExternalOutput")

    def internal(name, shape, shared=False, dt=F16):
        return nc.dram_tensor(name, list(shape), dt,
                              addr_space="Shared" if shared else "Local")

    u_shards = [u_shard0]
    it_shards = [it_shard0]
    fsU = {}     # l -> (agin [US,128], table [UPAD,128])
    fsI = {}     # l -> (agin [IS,128], table [IPAD,128])
    fdU = {}     # l -> local [US,128] = [fd_rb | fd_tr]
    for l in range(L):
        fsU[l] = (internal(f"agin_fsU{l}", [US, D2]),
                  internal(f"fsU{l}", [UPAD, D2]))
        fsI[l] = (internal(f"agin_fsI{l}", [IS, D2]),
                  internal(f"fsI{l}", [IPAD, D2]))
        fdU[l] = internal(f"fdU{l}", [US, D2])
        u_shards.append(internal(f"u{l + 1}", [US, D]))
        it_shards.append(internal(f"it{l + 1}", [IS, D]))
    q_sh = internal("q_sh", [US, D])
    p_sh = internal("p_sh", [US, D])
    hu_sh = internal("hu_sh", [US, PD])
    hi_sh = internal("hi_sh", [IS, PD])
    hu_t = internal("hu", [UPAD, PD])
    hi_t = internal("hi", [IPAD, PD])

    rg = [list(range(N_CORES))]

    with tile.TileContext(nc) as tc, ExitStack() as topctx:
        const = topctx.enter_context(tc.tile_pool(name="const", bufs=1))

        def cload(t, shape, dt):
            s = const.tile(list(shape), dt, tag=f"c_{t.name}")
            nc.sync.dma_start(out=s[:], in_=t.ap()[:, :])
            return s

        im = cload(iota_m_in, [P, P], F16)
        ident16 = cload(ident16_in, [P, P], F16)
        wu_sb = cload(wu, [D, L * 4 * D], F16)
        bu_sb = cload(bu, [P, L * 4 * D], F16)
        wi_sb = cload(wi, [D, L * 2 * D], F16)
        bi_sb = cload(bi_, [P, L * 2 * D], F16)
        a_sb = {nm: cload(a_in[nm], [P, L * D], F16) for nm in a_in}
        w1u_sb = cload(w1u, [D, L * 2 * D], F16)
        w1p_sb = cload(w1p, [D, L * 2 * D], F16)
        b1_sb = cload(b1, [P, L * 2 * D], F16)
        w2_sb = cload(w2, [P, L * 2 * D], F16)
        b2_sb = cload(b2, [P, L * 2], F32)

        # ------------------------------------------------------------------
        def proj_phase(l):
            """Row-sharded f16 projections; AllGather fsU/fsI tables."""
            with ExitStack() as ctx:
                sb = ctx.enter_context(tc.tile_pool(name=f"proj{l}", bufs=2))
                pst = ctx.enter_context(
                    tc.tile_pool(name=f"projt{l}", bufs=3, space="PSUM"))
                ps = ctx.enter_context(
                    tc.tile_pool(name=f"projp{l}", bufs=2, space="PSUM"))

                BT = 4

                def do(shard_tab, n_tiles, w_sb_l, b_sb_l, ncols, outs):
                    # outs: list of (dst_tensor, col_lo, col_hi)
                    for t0 in range(0, n_tiles, BT):
                        bt = min(BT, n_tiles - t0)
                        src = sb.tile([P, BT * D], F16, tag="psrc")
                        nc.sync.dma_start(
                            out=src[:, :bt * D].rearrange("p (g d) -> p g d", d=D),
                            in_=shard_tab.ap()[t0 * P:(t0 + bt) * P, :]
                            .rearrange("(g p) d -> p g d", p=P))
                        mm = ps.tile([P, BT * ncols], F32, tag="pmm",
                                     space="PSUM")
                        for k in range(bt):
                            tp = pst.tile([D, P], F16, tag="ptp", space="PSUM")
                            nc.tensor.transpose(
                                out=tp[:], in_=src[:, k * D:(k + 1) * D],
                                identity=ident16[:])
                            uT = sb.tile([D, P], F16, tag="puT")
                            nc.scalar.activation(uT[:], tp[:], Act.Copy)
                            nc.tensor.matmul(mm[:, k * ncols:(k + 1) * ncols],
                                             lhsT=uT[:], rhs=w_sb_l,
                                             start=True, stop=True)
                        big = sb.tile([P, BT * ncols], F16, tag="pbig")
                        nc.vector.tensor_tensor(
                            out=big[:, :bt * ncols]
                            .rearrange("p (g d) -> p g d", d=ncols),
                            in0=mm[:, :bt * ncols]
                            .rearrange("p (g d) -> p g d", d=ncols),
                            in1=b_sb_l.rearrange("p (g d) -> p g d", g=1)
                            .to_broadcast([P, bt, ncols]),
                            op=Alu.add)
                        for (dt_, lo, hi) in outs:
                            nc.sync.dma_start(
                                out=dt_.ap()[t0 * P:(t0 + bt) * P, :]
                                .rearrange("(g p) d -> p g d", p=P),
                                in_=big[:, :bt * ncols]
                                .rearrange("p (g d) -> p g d", d=ncols)[:, :, lo:hi])

                do(u_shards[l], UT,
                   wu_sb[:, l * 4 * D:(l + 1) * 4 * D],
                   bu_sb[:, l * 4 * D:(l + 1) * 4 * D], 4 * D,
                   [(fsU[l][0], 0, D2),        # [fs_rate | fs_tr]
                    (fdU[l], D2, 2 * D2)])     # [fd_rb | fd_tr]
                do(it_shards[l], IT,
                   wi_sb[:, l * 2 * D:(l + 1) * 2 * D],
                   bi_sb[:, l * 2 * D:(l + 1) * 2 * D], 2 * D,
                   [(fsI[l][0], 0, D2)])       # [fs_rb | fd_rate]

            import os as _os3
            if _os3.environ.get("KNOAG") == "1":
                return
            for ai, ao in (fsI[l], fsU[l]):
                nc.gpsimd.collective_compute(
                    "AllGather", Alu.bypass, replica_groups=rg,
                    ins=[ai.ap()[:, :]], outs=[ao.ap()[:, :]])

        # ------------------------------------------------------------------
        def gat_phase(l, g, fs_table, fs_lo, fd_table, fd_lo, den_first,
                      out_tensor, resid_tab):
            """Edge processing for one GAT; writes out_tensor [S, D] f16.

            fs_lo: column offset of this GAT's fs payload in the 128-col
            gathered row. den_first: ones column sits just BEFORE the fs
            payload (tr) instead of just after (rate/rb)."""
            import os as _os
            KG = int(_os.environ.get("KG", "9"))
            KFDE = _os.environ.get("KFDE", "1") == "1"
            gi = g_in[g.name]
            a_l = a_sb[g.name][:, l * D:(l + 1) * D]
            table_rows = fs_table.ap().shape[0]
            ones_col = fs_lo - 1 if den_first else fs_lo + D
            rhs_lo = fs_lo - 1 if den_first else fs_lo
            den_off = 0 if den_first else D
            num_off = 1 if den_first else 0
            with ExitStack() as ctx:
                sb = ctx.enter_context(tc.tile_pool(name=f"e{g.name}{l}", bufs=2))
                qp = ctx.enter_context(tc.tile_pool(name=f"eq{g.name}{l}", bufs=6))
                ps_acc = ctx.enter_context(
                    tc.tile_pool(name=f"ea{g.name}{l}", bufs=2, space="PSUM"))
                if KFDE:
                    ps_x = ctx.enter_context(
                        tc.tile_pool(name=f"ex{g.name}{l}", bufs=2, space="PSUM"))
                    ps_t = ctx.enter_context(
                        tc.tile_pool(name=f"et{g.name}{l}", bufs=2, space="PSUM"))

                K, Kb, nb = g.K, g.Kb, g.nb
                w_base = 0
                g_base = 0       # global sub-tile counter
                for wbi in g.blocks:
                    G = wbi * K  # sub-tiles in block
                    # loads
                    idx_t = sb.tile([P, (g.WB * K * P) // 16], I16, tag="idx")
                    c0 = g_base * P // 16
                    nc.sync.dma_start(
                        out=idx_t[:, :G * P // 16],
                        in_=gi["idx"].ap()[:, c0:c0 + G * P // 16])
                    win_of = []
                    for b in range(nb):
                        for wo in range(wbi):
                            win_of += [wo] * Kb[b]
                    if not KFDE:
                        idf_t = sb.tile([P, (g.WB * K * P) // 16], I16, tag="idf")
                        nc.sync.dma_start(
                            out=idf_t[:, :G * P // 16],
                            in_=gi["idxfd"].ap()[:, c0:c0 + G * P // 16])
                    else:
                        fd_t = sb.tile([P, g.WB * D], F16, tag="fdt")
                        nc.sync.dma_start(
                            out=fd_t[:, :wbi * D],
                            in_=fd_table.ap()[w_base * P:(w_base + wbi) * P,
                                              fd_lo:fd_lo + D]
                            .rearrange("(g p) d -> p g d", p=P))
                    dlc_t = sb.tile([P, g.WB * K], F32, tag="dlc")
                    nc.sync.dma_start(out=dlc_t[:, :G],
                                      in_=gi["dlc"].ap()[:, g_base:g_base + G])
                    if resid_tab is not None:
                        rs_t = sb.tile([P, g.WB * D], F16, tag="rs")
                        nc.sync.dma_start(
                            out=rs_t[:, :wbi * D],
                            in_=resid_tab.ap()[w_base * P:(w_base + wbi) * P, :]
                            .rearrange("(g p) d -> p g d", p=P))

                    fsg = sb.tile([P, g.WB * K * D2], F16, tag="fsg")
                    # fs gathers per bank
                    scol = 0
                    sg = 0
                    for b in range(nb):
                        ngb = wbi * Kb[b]          # sub-tiles for this bank
                        nidx = ngb * P
                        hi_row = min(table_rows, (b + 1) * BANK)
                        nc.gpsimd.dma_gather(
                            fsg[:, sg * D2:(sg + ngb) * D2]
                            .rearrange("p (g d) -> p g d", d=D2),
                            fs_table.ap()[b * BANK:hi_row, :],
                            idx_t[:, scol:scol + nidx // 16],
                            nidx, nidx, D2, single_packet=SPKT,
                            queue_num=_next_q())
                        scol += nidx // 16
                        sg += ngb
                    if not KFDE:
                        # fd gather (single range, local table)
                        fdg = sb.tile([P, g.WB * K * D2], F16, tag="fdg")
                        nc.gpsimd.dma_gather(
                            fdg[:, :G * D2].rearrange("p (g d) -> p g d", d=D2),
                            fd_table.ap()[:, :],
                            idf_t[:, :G * P // 16],
                            G * P, G * P, D2, single_packet=SPKT,
                            queue_num=_next_q())
                    # ones column for the denominator
                    nc.vector.memset(
                        fsg[:, :G * D2]
                        .rearrange("p (g d) -> p g d", d=D2)[:, :, ones_col:ones_col + 1],
                        1.0)

                    if KG <= 1:
                        out_t = sb.tile([P, g.WB * D], F16, tag="out")
                        nc.vector.tensor_tensor(
                            out=out_t[:, :wbi * D],
                            in0=fsg[:, :wbi * D],
                            in1=(fdg if not KFDE else fsg)[:, :wbi * D],
                            op=Alu.add)
                        nc.sync.dma_start(
                            out=out_tensor.ap()[w_base * P:(w_base + wbi) * P, :]
                            .rearrange("(g p) d -> p g d", p=P),
                            in_=out_t[:, :wbi * D].rearrange("p (g d) -> p g d", d=D))
                        w_base += wbi
                        g_base += G
                        continue

                    # X-stage: x = fs+fd, leaky, e = a.x, z = exp(e)
                    z_all = sb.tile([P, g.WB * K], F32, tag="zall")
                    e_all = sb.tile([P, g.WB * K], F32, tag="eall")
                    XG = 16
                    for x0 in range(0, G, XG):
                        xc = min(XG, G - x0)
                        if KFDE:
                            # x = onehot-expanded fd + fs, in PSUM via PE
                            xps = ps_x.tile([P, 16 * D], F32, tag="xps",
                                            space="PSUM")
                            for j in range(xc):
                                gg = x0 + j
                                q_t = qp.tile([P, P], F16, tag="qt")
                                nc.vector.tensor_scalar(
                                    out=q_t[:], in0=im[:],
                                    scalar1=dlc_t[:, gg:gg + 1],
                                    scalar2=None, op0=Alu.is_equal)
                                tp = ps_t.tile([P, P], F16, tag="tp",
                                               space="PSUM")
                                nc.tensor.transpose(out=tp[:], in_=q_t[:],
                                                    identity=ident16[:])
                                qtS = qp.tile([P, P], F16, tag="qtS")
                                if j % 2 == 0:
                                    nc.scalar.activation(qtS[:], tp[:], Act.Copy)
                                else:
                                    nc.vector.tensor_copy(out=qtS[:], in_=tp[:])
                                nc.tensor.matmul(
                                    xps[:, j * D:(j + 1) * D], lhsT=qtS[:],
                                    rhs=fd_t[:, win_of[gg] * D:(win_of[gg] + 1) * D],
                                    start=True, stop=False)
                                nc.tensor.matmul(
                                    xps[:, j * D:(j + 1) * D], lhsT=ident16[:],
                                    rhs=fsg[:, gg * D2 + fs_lo:gg * D2 + fs_lo + D],
                                    start=False, stop=True)
                            xl = sb.tile([P, 16 * D], F16, tag="xl")
                            nc.scalar.activation(xl[:, :xc * D],
                                                 xps[:, :xc * D], Act.Lrelu,
                                                 alpha=GAT_SLOPE)
                        else:
                            xb = sb.tile([P, 16 * D], F16, tag="xb")
                            nc.vector.tensor_tensor(
                                out=xb[:, :xc * D],
                                in0=fsg[:, x0 * D2:(x0 + xc) * D2]
                                .rearrange("p (g d) -> p g d", d=D2)[:, :, fs_lo:fs_lo + D],
                                in1=fdg[:, x0 * D2:(x0 + xc) * D2]
                                .rearrange("p (g d) -> p g d", d=D2)[:, :, fd_lo:fd_lo + D],
                                op=Alu.add)
                            xs = sb.tile([P, 16 * D], F16, tag="xs")
                            nc.vector.tensor_scalar_mul(
                                xs[:, :xc * D], xb[:, :xc * D], GAT_SLOPE)
                            xl = sb.tile([P, 16 * D], F16, tag="xl")
                            nc.vector.tensor_tensor(
                                out=xl[:, :xc * D], in0=xb[:, :xc * D],
                                in1=xs[:, :xc * D], op=Alu.max)
                        xa = sb.tile([P, 16 * D], F16, tag="xa")
                        nc.vector.tensor_tensor(
                            out=xa[:, :xc * D], in0=xl[:, :xc * D],
                            in1=a_l.rearrange("p (g d) -> p g d", g=1)
                            .to_broadcast([P, xc, D]),
                            op=Alu.mult)
                        nc.vector.reduce_sum(
                            out=e_all[:, x0:x0 + xc],
                            in_=xa[:, :xc * D].rearrange("p (g d) -> p g d", d=D),
                            axis=mybir.AxisListType.X)
                    # one Exp per block: minimizes Act func-table reloads
                    nc.scalar.activation(z_all[:, :G], e_all[:, :G], Act.Exp)

                    if KG <= 3:
                        out_t = sb.tile([P, g.WB * D], F16, tag="out")
                        nc.vector.tensor_copy(out=out_t[:, :wbi * D],
                                              in_=fsg[:, :wbi * D])
                        nc.sync.dma_start(
                            out=out_tensor.ap()[w_base * P:(w_base + wbi) * P, :]
                            .rearrange("(g p) d -> p g d", p=P),
                            in_=out_t[:, :wbi * D].rearrange("p (g d) -> p g d", d=D))
                        w_base += wbi
                        g_base += G
                        continue

                    # accumulation (window-major; PSUM groups sequential)
                    acc = ps_acc.tile([P, g.WB * (D + 1)], F32, tag="acc",
                                      space="PSUM")
                    for wo in range(wbi):
                        subs = [gg for gg in range(G) if win_of[gg] == wo]
                        for si, gg in enumerate(subs):
                            qts = qp.tile([P, P], F16, tag="qts")
                            nc.vector.tensor_scalar(
                                out=qts[:], in0=im[:],
                                scalar1=dlc_t[:, gg:gg + 1],
                                scalar2=z_all[:, gg:gg + 1],
                                op0=Alu.is_equal, op1=Alu.mult)
                            nc.tensor.matmul(
                                acc[:, wo * (D + 1):(wo + 1) * (D + 1)],
                                lhsT=qts[:],
                                rhs=fsg[:, gg * D2 + rhs_lo:gg * D2 + rhs_lo + D + 1],
                                start=(si == 0),
                                stop=(si == len(subs) - 1))

                    # divide + residual + store
                    out_t = sb.tile([P, g.WB * D], F16, tag="out")
                    den = sb.tile([P, g.WB], F32, tag="den")
                    nc.vector.tensor_scalar_max(
                        den[:, :wbi],
                        acc[:, :wbi * (D + 1)]
                        .rearrange("p (g d) -> p g d", d=D + 1)[:, :, den_off:den_off + 1],
                        1e-30)
                    rec = sb.tile([P, g.WB], F32, tag="rec")
                    nc.vector.reciprocal(rec[:, :wbi], den[:, :wbi])
                    for wo in range(wbi):
                        if resid_tab is None:
                            nc.vector.tensor_scalar_mul(
                                out_t[:, wo * D:(wo + 1) * D],
                                acc[:, wo * (D + 1) + num_off:wo * (D + 1) + num_off + D],
                                rec[:, wo:wo + 1])
                        else:
                            tmp = sb.tile([P, D], F32, tag="dtmp")
                            nc.vector.tensor_scalar_mul(
                                tmp[:], acc[:, wo * (D + 1) + num_off:wo * (D + 1) + num_off + D],
                                rec[:, wo:wo + 1])
                            nc.vector.tensor_tensor(
                                out=out_t[:, wo * D:(wo + 1) * D],
                                in0=tmp[:], in1=rs_t[:, wo * D:(wo + 1) * D],
                                op=Alu.add)
                    nc.sync.dma_start(
                        out=out_tensor.ap()[w_base * P:(w_base + wbi) * P, :]
                        .rearrange("(g p) d -> p g d", p=P),
                        in_=out_t[:, :wbi * D].rearrange("p (g d) -> p g d", d=D))

                    w_base += wbi
                    g_base += G

        # ------------------------------------------------------------------
        def epilogue_phase(l):
            with ExitStack() as ctx:
                sb = ctx.enter_context(tc.tile_pool(name=f"ep{l}", bufs=2))
                pst = ctx.enter_context(
                    tc.tile_pool(name=f"ept{l}", bufs=3, space="PSUM"))
                psm = ctx.enter_context(
                    tc.tile_pool(name=f"epm{l}", bufs=2, space="PSUM"))
                BT = 8
                w1u_l = w1u_sb[:, l * 2 * D:(l + 1) * 2 * D]
                w1p_l = w1p_sb[:, l * 2 * D:(l + 1) * 2 * D]
                b1_l = b1_sb[:, l * 2 * D:(l + 1) * 2 * D]
                w2_l = w2_sb[:, l * 2 * D:(l + 1) * 2 * D]
                b2_l = b2_sb[:, l * 2:(l + 1) * 2]
                for t0 in range(0, UT, BT):
                    bt = min(BT, UT - t0)
                    rows = slice(t0 * P, (t0 + bt) * P)
                    ut = sb.tile([P, BT * D], F16, tag="eu")
                    nc.sync.dma_start(
                        out=ut[:, :bt * D].rearrange("p (g d) -> p g d", d=D),
                        in_=u_shards[l].ap()[rows, :].rearrange("(g p) d -> p g d", p=P))
                    pt = sb.tile([P, BT * D], F16, tag="epp")
                    nc.sync.dma_start(
                        out=pt[:, :bt * D].rearrange("p (g d) -> p g d", d=D),
                        in_=p_sh.ap()[rows, :].rearrange("(g p) d -> p g d", p=P))
                    qt_ = sb.tile([P, BT * D], F16, tag="epq")
                    nc.sync.dma_start(
                        out=qt_[:, :bt * D].rearrange("p (g d) -> p g d", d=D),
                        in_=q_sh.ap()[rows, :].rearrange("(g p) d -> p g d", p=P))
                    # transposes (features on partitions) for u, p, q
                    trs = {}
                    for nm, srcp in (("u", ut), ("p", pt), ("q", qt_)):
                        big = sb.tile([D, BT * P], F16, tag=f"eT{nm}")
                        for k in range(bt):
                            tp = pst.tile([D, P], F16, tag="etp", space="PSUM")
                            nc.tensor.transpose(
                                out=tp[:], in_=srcp[:, k * D:(k + 1) * D],
                                identity=ident16[:])
                            nc.scalar.activation(big[:, k * P:(k + 1) * P],
                                                 tp[:], Act.Copy)
                        trs[nm] = big
                    # split-W1 matmuls, PSUM-accumulated: s1 = uT@W1u + (p|q)T@W1p
                    s_ps = {}
                    for ci, (nm2, col) in enumerate((("p", 0), ("q", 1))):
                        mm = psm.tile([P, BT * D], F32, tag=f"emm{ci}",
                                      space="PSUM")
                        for k in range(bt):
                            nc.tensor.matmul(
                                mm[:, k * D:(k + 1) * D],
                                lhsT=trs["u"][:, k * P:(k + 1) * P],
                                rhs=w1u_l[:, col * D:(col + 1) * D],
                                start=True, stop=False)
                            nc.tensor.matmul(
                                mm[:, k * D:(k + 1) * D],
                                lhsT=trs[nm2][:, k * P:(k + 1) * P],
                                rhs=w1p_l[:, col * D:(col + 1) * D],
                                start=False, stop=True)
                        s_ps[col] = mm
                    # batched vector tail: bias, leaky, .w2, reduce, +b2, leaky
                    s2 = []
                    for col in (0, 1):
                        s1 = sb.tile([P, BT * D], F16, tag=f"es1_{col}")
                        nc.vector.tensor_tensor(
                            out=s1[:, :bt * D].rearrange("p (g d) -> p g d", d=D),
                            in0=s_ps[col][:, :bt * D].rearrange("p (g d) -> p g d", d=D),
                            in1=b1_l[:, col * D:(col + 1) * D]
                            .rearrange("p (g d) -> p g d", g=1)
                            .to_broadcast([P, bt, D]),
                            op=Alu.add)
                        s1s = sb.tile([P, BT * D], F16, tag=f"es1s_{col}")
                        nc.vector.tensor_scalar_mul(
                            s1s[:, :bt * D], s1[:, :bt * D], MLP_SLOPE)
                        s1l = sb.tile([P, BT * D], F16, tag=f"es1l_{col}")
                        nc.vector.tensor_tensor(
                            out=s1l[:, :bt * D], in0=s1[:, :bt * D],
                            in1=s1s[:, :bt * D], op=Alu.max)
                        xw = sb.tile([P, BT * D], F16, tag=f"exw_{col}")
                        nc.vector.tensor_tensor(
                            out=xw[:, :bt * D].rearrange("p (g d) -> p g d", d=D),
                            in0=s1l[:, :bt * D].rearrange("p (g d) -> p g d", d=D),
                            in1=w2_l[:, col * D:(col + 1) * D]
                            .rearrange("p (g d) -> p g d", g=1)
                            .to_broadcast([P, bt, D]),
                            op=Alu.mult)
                        sv = sb.tile([P, BT], F32, tag=f"esv_{col}")
                        nc.vector.reduce_sum(
                            out=sv[:, :bt],
                            in_=xw[:, :bt * D].rearrange("p (g d) -> p g d", d=D),
                            axis=mybir.AxisListType.X)
                        svb = sb.tile([P, BT], F32, tag=f"esvb_{col}")
                        nc.vector.tensor_scalar_add(
                            svb[:, :bt], sv[:, :bt], b2_l[:, col:col + 1])
                        svs = sb.tile([P, BT], F32, tag=f"esvs_{col}")
                        nc.vector.tensor_scalar_mul(
                            svs[:, :bt], svb[:, :bt], MLP_SLOPE)
                        svl = sb.tile([P, BT], F32, tag=f"esvl_{col}")
                        nc.vector.tensor_tensor(
                            out=svl[:, :bt], in0=svb[:, :bt],
                            in1=svs[:, :bt], op=Alu.max)
                        s2.append(svl)
                    dg = sb.tile([P, BT], F32, tag="edg")
                    nc.vector.tensor_tensor(
                        out=dg[:, :bt], in0=s2[0][:, :bt], in1=s2[1][:, :bt],
                        op=Alu.subtract)
                    g0 = sb.tile([P, BT], F16, tag="eg0")
                    nc.scalar.activation(g0[:, :bt], dg[:, :bt], Act.Sigmoid)
                    # out = u + q + g0*(p - q)
                    pq = sb.tile([P, BT * D], F16, tag="epq2")
                    nc.vector.tensor_tensor(
                        out=pq[:, :bt * D], in0=pt[:, :bt * D],
                        in1=qt_[:, :bt * D], op=Alu.subtract)
                    gpq = sb.tile([P, BT * D], F16, tag="egpq")
                    nc.vector.tensor_tensor(
                        out=gpq[:, :bt * D].rearrange("p (g d) -> p g d", d=D),
                        in0=pq[:, :bt * D].rearrange("p (g d) -> p g d", d=D),
                        in1=g0[:, :bt].rearrange("p (g d) -> p g d", d=1)
                        .to_broadcast([P, bt, D]),
                        op=Alu.mult)
                    uq = sb.tile([P, BT * D], F16, tag="euq")
                    nc.vector.tensor_tensor(
                        out=uq[:, :bt * D], in0=ut[:, :bt * D],
                        in1=qt_[:, :bt * D], op=Alu.add)
                    ot = sb.tile([P, BT * D], F16, tag="eo")
                    nc.vector.tensor_tensor(
                        out=ot[:, :bt * D], in0=uq[:, :bt * D],
                        in1=gpq[:, :bt * D], op=Alu.add)
                    nc.sync.dma_start(
                        out=u_shards[l + 1].ap()[rows, :]
                        .rearrange("(g p) d -> p g d", p=P),
                        in_=ot[:, :bt * D].rearrange("p (g d) -> p g d", d=D))

        # ------------------------------------------------------------------
        def hu_build_phase(which):
            """Assemble hu_sh [US, PD] / hi_sh f16 locally, then AllGather."""
            with ExitStack() as ctx:
                sb = ctx.enter_context(tc.tile_pool(name=f"hub{which}", bufs=2))
                BT = 16
                for shards, out_tab, n_tiles in (((u_shards, hu_sh, UT),)
                                                 if which == "u" else
                                                 ((it_shards, hi_sh, IT),)):
                    for t0 in range(0, n_tiles, BT):
                        bt = min(BT, n_tiles - t0)
                        rows = slice(t0 * P, (t0 + bt) * P)
                        big = sb.tile([P, BT * PD], F16, tag="hbig")
                        nc.vector.memset(
                            big[:, :bt * PD]
                            .rearrange("p (g d) -> p g d", d=PD)
                            [:, :, (L + 1) * D:PD], 0)
                        for li, tab in enumerate(shards):
                            ld = sb.tile([P, BT * D], F16, tag="hld")
                            nc.sync.dma_start(
                                out=ld[:, :bt * D].rearrange("p (g d) -> p g d", d=D),
                                in_=tab.ap()[rows, :]
                                .rearrange("(g p) d -> p g d", p=P))
                            nc.vector.tensor_copy(
                                out=big[:, :bt * PD]
                                .rearrange("p (g d) -> p g d", d=PD)
                                [:, :, li * D:(li + 1) * D],
                                in_=ld[:, :bt * D]
                                .rearrange("p (g d) -> p g d", d=D))
                        nc.sync.dma_start(
                            out=out_tab.ap()[rows, :]
                            .rearrange("(g p) d -> p g d", p=P),
                            in_=big[:, :bt * PD].rearrange("p (g d) -> p g d", d=PD))
            import os as _os4
            if _os4.environ.get("KNOAG") == "1":
                return
            ai, ao = (hu_sh, hu_t) if which == "u" else (hi_sh, hi_t)
            nc.gpsimd.collective_compute(
                "AllGather", Alu.bypass, replica_groups=rg,
                ins=[ai.ap()[:, :]], outs=[ao.ap()[:, :]])

        # ------------------------------------------------------------------
        def pred_phase():
            with ExitStack() as ctx:
                sb = ctx.enter_context(tc.tile_pool(name="pred", bufs=2))
                G = pred.G_blk
                for bi in range(pred.n_blocks):
                    hu_g = sb.tile([P, G * PD], F16, tag="phu")
                    hi_g = sb.tile([P, G * PD], F16, tag="phi")
                    iu_t = sb.tile([P, G * P // 16], I16, tag="piu")
                    c0 = bi * G * P // 16
                    nc.sync.dma_start(out=iu_t[:],
                                      in_=pidxu.ap()[:, c0:c0 + G * P // 16])
                    ii_t = sb.tile([P, G * P // 16], I16, tag="pii")
                    nc.sync.dma_start(out=ii_t[:],
                                      in_=pidxi.ap()[:, c0:c0 + G * P // 16])
                    # hi gathers first: hi_t is AllGathered early, so these
                    # overlap the trailing hu AllGather on the in-order queue
                    sg = 0
                    scol = 0
                    for u_ in range(pred.nbu):
                        for i_ in range(pred.nbi):
                            ngb = pred.Kp[(u_, i_)]
                            nidx = ngb * P
                            hi_row = min(hi_t.ap().shape[0], (i_ + 1) * BANK)
                            nc.gpsimd.dma_gather(
                                hi_g[:, sg * PD:(sg + ngb) * PD]
                                .rearrange("p (g d) -> p g d", d=PD),
                                hi_t.ap()[i_ * BANK:hi_row, :],
                                ii_t[:, scol:scol + nidx // 16],
                                nidx, nidx, PD, single_packet=SPKT,
                                queue_num=_next_q())
                            sg += ngb
                            scol += nidx // 16
                    # hu gathers: per user bank (spans its item-bank pairs)
                    sg = 0
                    scol = 0
                    for u_ in range(pred.nbu):
                        ngb = sum(pred.Kp[(u_, i_)] for i_ in range(pred.nbi))
                        nidx = ngb * P
                        hi_row = min(hu_t.ap().shape[0], (u_ + 1) * BANK)
                        nc.gpsimd.dma_gather(
                            hu_g[:, sg * PD:(sg + ngb) * PD]
                            .rearrange("p (g d) -> p g d", d=PD),
                            hu_t.ap()[u_ * BANK:hi_row, :],
                            iu_t[:, scol:scol + nidx // 16],
                            nidx, nidx, PD, single_packet=SPKT,
                            queue_num=_next_q())
                        sg += ngb
                        scol += nidx // 16
                    # dots (batched f16 mult + per-group reduce)
                    dt_ = sb.tile([P, G], F32, tag="pdot")
                    for x0 in range(0, G, 8):
                        xc = min(8, G - x0)
                        prod = sb.tile([P, 8 * PD], F16, tag="pprod")
                        nc.vector.tensor_tensor(
                            out=prod[:, :xc * PD],
                            in0=hu_g[:, x0 * PD:(x0 + xc) * PD],
                            in1=hi_g[:, x0 * PD:(x0 + xc) * PD], op=Alu.mult)
                        nc.vector.reduce_sum(
                            out=dt_[:, x0:x0 + xc],
                            in_=prod[:, :xc * PD]
                            .rearrange("p (g d) -> p g d", d=PD),
                            axis=mybir.AxisListType.X)
                    nc.sync.dma_start(out=pred_out.ap()[:, bi * G:(bi + 1) * G],
                                      in_=dt_[:])

        # ------------------------------------------------------------------
        phase_order = []
        for l in range(L):
            phase_order += [f"proj{l}", f"rb{l}", f"rate{l}"]
            if l == L - 1:
                phase_order += ["hib"]
            phase_order += [f"tr{l}", f"epi{l}"]
        phase_order += ["hu", "pred"]

        global PHASE_MARKS
        PHASE_MARKS = []

        def run_until():
            for ph in phase_order:
                PHASE_MARKS.append((ph, nc.next_id()))
                l = int(ph[-1]) if ph[-1].isdigit() else 0
                if ph.startswith("proj"):
                    proj_phase(l)
                elif ph.startswith("rate"):
                    # fs from fsU cols 0:64, fd from fsI-agin cols 64:128
                    gat_phase(l, rate, fsU[l][1], 0, fsI[l][0], D, False,
                              it_shards[l + 1], it_shards[l])
                elif ph.startswith("rb"):
                    # fs from fsI cols 0:64, fd from fdU cols 0:64
                    gat_phase(l, rb, fsI[l][1], 0, fdU[l], 0, False,
                              q_sh, None)
                elif ph.startswith("tr"):
                    # fs from fsU cols 64:128, fd from fdU cols 64:128
                    gat_phase(l, tr, fsU[l][1], D, fdU[l], D, True,
                              p_sh, None)
                elif ph.startswith("epi"):
                    epilogue_phase(l)
                elif ph == "hib":
                    hu_build_phase("i")
                elif ph == "hu":
                    hu_build_phase("u")
                elif ph == "pred":
                    pred_phase()
                if ph == kphase:
                    return

        run_until()
        if dbg_out is not None:
            dbg_tensors = dict(
                q_sh=q_sh, p_sh=p_sh, hu=hu_t, hi=hi_t, hu_sh=hu_sh,
                hi_sh=hi_sh,
                **{f"u_shard{i}": t for i, t in enumerate(u_shards)},
                **{f"it_shard{i}": t for i, t in enumerate(it_shards)},
                **{f"fsU{l}": fsU[l][1] for l in range(L)},
                **{f"fsI{l}": fsI[l][1] for l in range(L)},
                **{f"agin_fsU{l}": fsU[l][0] for l in range(L)},
                **{f"agin_fsI{l}": fsI[l][0] for l in range(L)},
                **{f"fdU{l}": fdU[l] for l in range(L)},
            )
            src_t = dbg_tensors[dbg_spec[0]]
            sdt = src_t.ap().dtype
            with ExitStack() as ctx:
                sbd = ctx.enter_context(tc.tile_pool(name="dbg", bufs=2))
                rows, cols = dbg_spec[1], dbg_spec[2]
                for r0 in range(0, rows, P):
                    rc = min(P, rows - r0)
                    t_ = sbd.tile([P, cols], sdt, tag="dbg")
                    nc.sync.dma_start(out=t_[:rc, :],
                                      in_=src_t.ap()[r0:r0 + rc, :])
                    if sdt != F32:
                        t2 = sbd.tile([P, cols], F32, tag="dbg2")
                        nc.vector.tensor_copy(out=t2[:rc, :], in_=t_[:rc, :])
                        t_ = t2
                    nc.sync.dma_start(out=dbg_out.ap()[r0:r0 + rc, :],
                                      in_=t_[:rc, :])

    nc.compile()
    return nc


# ---------------------------------------------------------------------------
# entry point
# ---------------------------------------------------------------------------

def _pad_rows(a, rows):
    out = np.zeros((rows, a.shape[1]), dtype=a.dtype)
    out[:a.shape[0]] = a
    return out


def kernel(**inputs):
    U, D = inputs["user_emb"].shape
    I = inputs["item_emb"].shape[0]
    L = inputs["rate_Ws"].shape[0]
    UT = _ceil(_ceil(U, P), N_CORES)
    IT = _ceil(_ceil(I, P), N_CORES)
    US, IS = UT * P, IT * P
    UPAD, IPAD = US * N_CORES, IS * N_CORES
    # gather elem size must be a multiple of 256 bytes -> PD*2 % 256 == 0
    PD = _ceil(D * (L + 1) * 2, 256) * 128

    rate_src = np.asarray(inputs["rate_src"])
    rate_dst = np.asarray(inputs["rate_dst"])
    trust_src = np.asarray(inputs["trust_src"])
    trust_dst = np.asarray(inputs["trust_dst"])

    rate = GatStruct("rate", rate_src, rate_dst, UPAD, IT)
    rb = GatStruct("rb", rate_dst, rate_src, IPAD, UT)
    tr = GatStruct("tr", trust_src, trust_dst, UPAD, UT)

    pos_src = np.asarray(inputs["pos_src"])
    pos_dst = np.asarray(inputs["pos_dst"])
    neg_src = np.asarray(inputs["neg_src"])
    neg_dst = np.asarray(inputs["neg_dst"])
    psrc = np.concatenate([pos_src, neg_src])
    pdst = np.concatenate([pos_dst, neg_dst])
    pred = PredStruct(psrc, pdst, UPAD, IPAD, block_edges=6144)

    import os
    hp = dict(U=U, I=I, D=D, L=L, UT=UT, IT=IT, PD=PD,
              rate=rate, rb=rb, tr=tr, pred=pred)
    print(f"[kernel] struct: rate K={rate.K} Kb={rate.Kb} WB={rate.WB} blocks={len(rate.blocks)}; "
          f"rb K={rb.K} WB={rb.WB} blocks={len(rb.blocks)}; "
          f"tr K={tr.K} WB={tr.WB} blocks={len(tr.blocks)}; "
          f"pred G_blk={pred.G_blk} blocks={pred.n_blocks}")
    kdbg = os.environ.get("KDBG")
    if kdbg:
        shp = {}
        for i in range(L + 1):
            shp[f"u_shard{i}"] = (US, D); shp[f"it_shard{i}"] = (IS, D)
        for l in range(L):
            shp[f"fsU{l}"] = (UPAD, 2 * D); shp[f"fsI{l}"] = (IPAD, 2 * D)
            shp[f"agin_fsU{l}"] = (US, 2 * D); shp[f"agin_fsI{l}"] = (IS, 2 * D)
            shp[f"fdU{l}"] = (US, 2 * D)
        shp["q_sh"] = (US, D); shp["p_sh"] = (US, D)
        shp["hu"] = (UPAD, PD); shp["hi"] = (IPAD, PD)
        shp["hu_sh"] = (US, PD); shp["hi_sh"] = (IS, PD)
        hp["dbg_spec"] = (kdbg, *shp[kdbg])

    t_b = __import__("time").time()
    nc = build_program(hp)
    print(f"[kernel] build+compile: {__import__('time').time() - t_b:.1f}s")

    # ---- inputs ----
    f16 = NPF16
    ue_pad = _pad_rows(inputs["user_emb"], UPAD).astype(f16)
    ie_pad = _pad_rows(inputs["item_emb"], IPAD).astype(f16)
    wu = np.concatenate([
        np.concatenate([inputs["rate_Ws"][l], inputs["tr_Ws"][l],
                        inputs["rb_Wd"][l], inputs["tr_Wd"][l]], axis=1)
        for l in range(L)], axis=1).astype(f16)
    bu = np.concatenate([
        np.tile(np.concatenate([inputs["rate_bs"][l], inputs["tr_bs"][l],
                                inputs["rb_bd"][l], inputs["tr_bd"][l]])[None, :],
                (P, 1))
        for l in range(L)], axis=1).astype(f16)
    wi = np.concatenate([
        np.concatenate([inputs["rb_Ws"][l], inputs["rate_Wd"][l]], axis=1)
        for l in range(L)], axis=1).astype(f16)
    bi_ = np.concatenate([
        np.tile(np.concatenate([inputs["rb_bs"][l], inputs["rate_bd"][l]])[None, :],
                (P, 1))
        for l in range(L)], axis=1).astype(f16)
    a_arrs = {}
    for nm in ("rate", "rb", "tr"):
        a_arrs[nm] = np.concatenate([
            np.tile(np.asarray(inputs[f"{nm}_a"][l])[None, :], (P, 1))
            for l in range(L)], axis=1).astype(f16)
    w1u = np.concatenate([
        np.concatenate([inputs["inf_W1"][l][:D], inputs["int_W1"][l][:D]],
                       axis=1)
        for l in range(L)], axis=1).astype(f16)
    w1p = np.concatenate([
        np.concatenate([inputs["inf_W1"][l][D:], inputs["int_W1"][l][D:]],
                       axis=1)
        for l in range(L)], axis=1).astype(f16)
    b1 = np.concatenate([
        np.tile(np.concatenate([inputs["inf_b1"][l], inputs["int_b1"][l]])[None, :],
                (P, 1))
        for l in range(L)], axis=1).astype(f16)
    w2 = np.concatenate([
        np.tile(np.concatenate([inputs["inf_W2"][l][:, 0],
                                inputs["int_W2"][l][:, 0]])[None, :], (P, 1))
        for l in range(L)], axis=1).astype(f16)
    b2 = np.concatenate([
        np.tile(np.array([[inputs["inf_b2"][l][0], inputs["int_b2"][l][0]]],
                         dtype=np.float32), (P, 1))
        for l in range(L)], axis=1).astype(np.float32)
    iota = np.arange(P, dtype=np.float32)
    iota_m = np.tile(iota[None, :], (P, 1)).astype(f16)
    ident16 = np.eye(P, dtype=f16)

    in_maps = []
    for c in range(N_CORES):
        m = {
            "u_shard0": ue_pad[c * US:(c + 1) * US],
            "it_shard0": ie_pad[c * IS:(c + 1) * IS],
            "wu": wu, "bu": bu, "wi": wi, "bi": bi_,
            "a_rate": a_arrs["rate"], "a_rb": a_arrs["rb"], "a_tr": a_arrs["tr"],
            "w1u": w1u, "w1p": w1p, "b1": b1, "w2": w2, "b2": b2,
            "iota_m": iota_m, "ident16": ident16,
            "pred_idxu": pred.idxu[c], "pred_idxi": pred.idxi[c],
        }
        for g in (rate, rb, tr):
            m[f"{g.name}_idx"] = g.idx16[c]
            m[f"{g.name}_idxfd"] = g.idxfd[c]
            m[f"{g.name}_dlc"] = g.dlc[c]
        in_maps.append(m)

    trace = os.environ.get("KTRACE") == "1"
    t_run = __import__("time").time()
    res = run_bass_kernel_spmd(nc, in_maps, core_ids=list(range(N_CORES)),
                               trace=trace)
    print(f"[kernel] device run wall: {__import__('time').time() - t_run:.1f}s")
    global LAST_RES, LAST_HP, LAST_EXEC_NS
    LAST_RES, LAST_HP, LAST_EXEC_NS = res, hp, res.exec_time_ns
    if os.environ.get("KBENCH") == "1":
        tmin = bench_pjrt(nc, in_maps, iters=int(os.environ.get("KBENCH_ITERS", "4")))
        LAST_EXEC_NS = int(tmin * 1e9)

    # ---- assemble outputs ----
    E = len(psrc)
    out = np.zeros((E,), dtype=np.float32)
    for c in range(N_CORES):
        vals = res.results[c]["pred_out"]  # [128, G_total]
        smap = pred.slotmap[c]
        gidx = np.arange(len(smap))
        v = vals[gidx % P, gidx // P]
        ok = smap >= 0
        out[smap[ok]] = v[ok]
    pos = out[:len(pos_src)].reshape(-1, 1)
    neg = out[len(pos_src):].reshape(-1, 1)
    return pos, neg
